# revision 1
# baseline (speedup 1.0000x reference)
"""Distributed GQA attention (llama-style RoPE) for one TRN2 chip (8 NeuronCores).

Sharding: core c handles batch b=c//4 and sequence-quarter q=c%4 (512 q-rows).
Each core projects Q for its own rows (all 32 heads), projects K/V for its own
512 positions, AllGathers K/V within its 4-core batch group, runs attention for
its rows, and applies the output projection. Output rows are disjoint across
cores, so no all-reduce is needed; the host concatenates.

On-chip dataflow (per core):
  xT   = x.T via PE transposes                      [d, rows]   f32
  kT   = wk.T @ xT   -> RoPE -> bf16 -> AllGather   [feat, skv]
  v    = xT.T @ wv   -> bf16 -> AllGather           [skv, feat]
  per head pair (with the previous group's out-proj interleaved):
    qT   = wq.T @ xT (f32r) -> RoPE -> bf16         [feat, sq]
    sT   = kT_h.T @ qT_h  (row-packed pairs)        [skv, sq]  psum f32
    e    = exp(sT/8) on ScalarE -> bf16
    oT   = [v|1x32].T @ e  accum over skv chunks    [96, sq]   psum f32
           (cols 64:96 are ones -> denominator lands replicated 32-wide)
    oT_n = oT[:64] * recip(oT[64:96])               -> f32r sbuf
  out  = oT_n.T @ wo  (f32r), 4 head-group partials summed in DRAM via DMA-CCE

All weights are pre-swizzled on the HOST into the exact SBUF layouts so every
DMA is a fully-linear copy. RoPE uses the half-rotated layout: wq/wk columns
are permuted on the host so each head's features are [evens(32) | odds(32)];
cos/sin tables are shipped pre-transposed/tiled as CC/SS [128, sq].
"""
import sys

sys.path.insert(0, "/opt/trn_rl_repo")

import numpy as np
import ml_dtypes
from contextlib import ExitStack

import concourse.bass as bass
import concourse.mybir as mybir
import concourse.tile as tile
from concourse import bacc
from concourse.bass_utils import run_bass_kernel_spmd
from concourse.masks import make_identity

B, S, D = 2, 2048, 2048
NQ, NKV, HD = 32, 8, 64
NCORES = 8
GPB = 4                 # cores per batch group
SQ = S // GPB           # 512 q-rows per core
P = 128
DC = D // P             # 16 contraction chunks
KF = NKV * HD           # 512 kv feature dim
KFC = KF // P           # 4 kv feature chunks
SC = S // P             # 16 skv chunks
RQ = SQ // P            # 4 q-row blocks
VW = HD + 32            # v_aug width: 64 v cols + 32 ones cols

FP = mybir.dt.float32
BF = mybir.dt.bfloat16
F32R = mybir.dt.float32r
EXPF = mybir.ActivationFunctionType.Exp
EXP_SCALE = 1.0 / 8.0   # 1/sqrt(HD)


def build(solo=False):
    nc = bacc.Bacc("TRN2", target_bir_lowering=False, debug=False,
                   num_devices=1 if solo else NCORES)

    x_e = nc.dram_tensor("x", [P, DC, SQ], BF, kind="ExternalInput").ap()
    wq_e = nc.dram_tensor("wq", [DC, P, DC, P], BF, kind="ExternalInput").ap()
    wk_e = nc.dram_tensor("wk", [P, DC, KF], BF, kind="ExternalInput").ap()
    wv_e = nc.dram_tensor("wv", [P, DC, KF], BF, kind="ExternalInput").ap()
    wo_e = nc.dram_tensor("wo", [RQ, RQ, P, RQ, 512], BF, kind="ExternalInput").ap()
    cc_e = nc.dram_tensor("cc", [P, SQ], FP, kind="ExternalInput").ap()
    ss_e = nc.dram_tensor("ss", [P, SQ], FP, kind="ExternalInput").ap()
    out_e = nc.dram_tensor("out", [SQ, D], FP, kind="ExternalOutput").ap()

    groups = [[0, 1, 2, 3], [4, 5, 6, 7]]

    with tile.TileContext(nc) as tc, ExitStack() as ctx:
        sb = ctx.enter_context(tc.tile_pool(name="sb", bufs=1))
        rp = ctx.enter_context(tc.tile_pool(name="rp", bufs=3))
        epool = ctx.enter_context(tc.tile_pool(name="epool", bufs=6))
        npool = ctx.enter_context(tc.tile_pool(name="npool", bufs=3))
        opool = ctx.enter_context(tc.tile_pool(name="opool", bufs=4))
        otp = ctx.enter_context(tc.tile_pool(name="otp", bufs=2))
        early = ctx.enter_context(tc.tile_pool(name="early", bufs=1))
        wqp = ctx.enter_context(tc.tile_pool(name="wqp", bufs=3))
        dram = ctx.enter_context(tc.tile_pool(name="dram", bufs=1, space="DRAM"))
        pp = ctx.enter_context(tc.tile_pool(name="pp", bufs=2, space="PSUM"))
        psc = ctx.enter_context(tc.tile_pool(name="psc", bufs=2, space="PSUM"))
        po = ctx.enter_context(tc.tile_pool(name="po", bufs=2, space="PSUM"))

        # ---- constants ----
        cc_sb = sb.tile([P, SQ], FP)
        ss_sb = sb.tile([P, SQ], FP)

        def rope_chunk(ps, dst):
            """dst = RoPE(ps) in half-rotated layout; ps [128,SQ] psum f32."""
            t0 = rp.tile([P, SQ], FP, tag="t0")
            t1 = rp.tile([P, SQ], FP, tag="t1")
            nc.vector.tensor_mul(t0[:], ps[:], cc_sb[:])
            for blk in range(4):
                o0, i0 = blk * 32, (blk ^ 1) * 32
                nc.vector.tensor_mul(t1[o0:o0 + 32, :], ps[i0:i0 + 32, :],
                                     ss_sb[o0:o0 + 32, :])
            nc.vector.tensor_add(dst, t0[:], t1[:])

        qT = sb.tile([P, DC, SQ], BF)
        kag_in = dram.tile([KF, SQ], BF)
        kag_out = dram.tile([GPB * KF, SQ], BF)
        vag_in = dram.tile([SQ, KF], BF)
        vag_out = dram.tile([S, KF], BF)

        # ---- pure input loads first (sync queue stays unblocked);
        #      x arrives pre-transposed from the host as xT [p, dc, s] ----
        xT = early.tile([P, DC, SQ], BF, tag="xT", name="xT")
        nc.sync.dma_start(xT[:], x_e)
        wkq = early.tile([P, DC, KF], BF, tag="wkq", name="wkq")
        nc.sync.dma_start(wkq[:], wk_e)
        wq_tiles = {}
        for pair in (0, 1):
            w = wqp.tile([P, DC, P], BF, tag="wq", name=f"wq_{pair}")
            nc.sync.dma_start(w[:], wq_e[pair])
            wq_tiles[pair] = w
        nc.sync.dma_start(cc_sb[:], cc_e)
        nc.sync.dma_start(ss_sb[:], ss_e)

        def qproj(pair):
            if pair in wq_tiles:
                wq_sb = wq_tiles.pop(pair)
            else:
                wq_sb = wqp.tile([P, DC, P], BF, tag="wq", name=f"wq_{pair}")
                nc.sync.dma_start(wq_sb[:], wq_e[pair])
            qps = pp.tile([P, 512], FP, tag="pp", name="qps")
            for dc in range(DC):
                nc.tensor.matmul(qps[:, :SQ], lhsT=wq_sb[:, dc, :],
                                 rhs=xT[:, dc, :],
                                 start=(dc == 0), stop=(dc == DC - 1))
            rope_chunk(qps[:, :SQ], qT[:, pair, :])

        # q-projection for the first two pairs fills the wk-load bubble
        qproj(0)
        qproj(1)

        # ---- K projection + RoPE -> AllGather ----
        kT_own = sb.tile([P, KFC, SQ], BF, tag="own4", name="kT_own")
        for fc in range(KFC):
            ps = pp.tile([P, 512], FP, tag="pp", name="kps")
            for dc in range(DC):
                nc.tensor.matmul(ps[:, :SQ],
                                 lhsT=wkq[:, dc, fc * P:(fc + 1) * P],
                                 rhs=xT[:, dc, :],
                                 start=(dc == 0), stop=(dc == DC - 1))
            rope_chunk(ps[:, :SQ], kT_own[:, fc, :])
        nc.sync.dma_start(kag_in[:].rearrange("(c p) s -> p c s", p=P),
                          kT_own[:])
        if solo:
            for r in range(GPB):
                nc.sync.dma_start(kag_out[r * KF:(r + 1) * KF, :], kag_in[:])
        else:
            nc.gpsimd.collective_compute(
                "AllGather", mybir.AluOpType.bypass, replica_groups=groups,
                ins=[kag_in[:]], outs=[kag_out[:]])

        # q-projection for pairs 2/3 fills the wv-load bubble
        qproj(2)
        qproj(3)

        # ---- V projection -> AllGather ----
        # wv reuses the x slot (x is dead once transposes finished)
        wv_sb = early.tile([P, DC, KF], BF, tag="big32", name="wv_sb")
        nc.sync.dma_start(wv_sb[:], wv_e)
        v_own = sb.tile([P, RQ, KF], BF, tag="own4", name="v_own")
        for pc in range(RQ):
            ps = pp.tile([P, 512], FP, tag="pp", name="vps")
            for dc in range(DC):
                nc.tensor.matmul(ps[:, :KF],
                                 lhsT=xT[:, dc, pc * P:(pc + 1) * P],
                                 rhs=wv_sb[:, dc, :],
                                 start=(dc == 0), stop=(dc == DC - 1))
            nc.vector.tensor_copy(v_own[:, pc, :], ps[:, :KF])
        nc.sync.dma_start(vag_in[:].rearrange("(c p) f -> p c f", p=P),
                          v_own[:])
        if solo:
            for r in range(GPB):
                nc.sync.dma_start(vag_out[r * SQ:(r + 1) * SQ, :], vag_in[:])
        else:
            nc.gpsimd.collective_compute(
                "AllGather", mybir.AluOpType.bypass, replica_groups=groups,
                ins=[vag_in[:]], outs=[vag_out[:]])

        # q-projection for pairs 4/5 fills the AllGather wait window
        qproj(4)
        qproj(5)

        # ---- land gathered K/V (kT reuses the wk slot) ----
        kT = early.tile([P, KFC, S], BF, tag="wkq", name="kT")
        for fc in range(KFC):
            for r in range(GPB):
                nc.scalar.dma_start(kT[:, fc, r * SQ:(r + 1) * SQ],
                                    kag_out[r * KF + fc * P: r * KF + (fc + 1) * P, :])
        v_aug = early.tile([P, NKV, SC, VW], BF, tag="big32", name="v_aug")
        nc.gpsimd.memset(v_aug[:], 1.0)
        for c in range(SC):
            nc.sync.dma_start(
                v_aug[:, :, c, 0:HD],
                vag_out[c * P:(c + 1) * P, :].rearrange("p (kv d) -> p kv d", d=HD))

        # ---- per-pair: Q proj + attention; prev group's out-proj interleaved ----
        oT_tiles = {}

        def wo_load(g, nf):
            wo_nf = opool.tile([P, 4, 512], BF, tag="wo", name="wo_nf")
            nc.sync.dma_start(wo_nf[:], wo_e[g, nf])
            return wo_nf

        out_acc = sb.tile([P, RQ, D], FP)

        def out_proj_m(g, nf, wo_nf, m):
            """Emit one [128-row, 512-col] tile of group g's out-projection,
            accumulated in SBUF; group 3 streams the finished slice out."""
            oT = oT_tiles[g]
            ps = pp.tile([P, 512], FP, tag="pp", name="ops")
            for ch in range(4):
                nc.tensor.matmul(ps[:],
                                 lhsT=oT[:, ch, m * P:(m + 1) * P],
                                 rhs=wo_nf[:, ch, :],
                                 start=(ch == 0), stop=(ch == 3))
            acc = out_acc[:, m, nf * 512:(nf + 1) * 512]
            if g == 0:
                nc.vector.tensor_copy(acc, ps[:])
            else:
                nc.vector.tensor_add(acc, acc, ps[:])
            if g == 3:
                nc.gpsimd.dma_start(
                    out_e[m * P:(m + 1) * P, nf * 512:(nf + 1) * 512], acc)

        for g in range(4):                    # 4 groups of 4 pairs
            oT_tiles[g] = otp.tile([P, RQ, SQ], BF, tag="oT", name=f"oT_{g}")
            for pi in range(4):               # pairs within group
                pair = g * 4 + pi
                wo_cur = [None]
                kc = pair % 4                 # kv chunk holding both kv heads
                kva, kvb = 2 * (pair % 4), 2 * (pair % 4) + 1


                psOA = po.tile([P, 512], FP, tag="po", name="psOA")
                psOB = po.tile([P, 512], FP, tag="po", name="psOB")
                eabs = {}
                for c in range(SC + 1):
                    if c < SC:
                        # scores for both heads of the pair into one 2-bank
                        # psum tile; one exp op covers A and B
                        psAB = psc.tile([P, 1024], FP, tag="psc", name="psAB")
                        nc.tensor.matmul(psAB[:, 0:SQ],
                                         lhsT=kT[0:64, kc, c * P:(c + 1) * P],
                                         rhs=qT[0:64, pair, :],
                                         start=True, stop=True,
                                         tile_position=(0, 0))
                        nc.tensor.matmul(psAB[:, SQ:2 * SQ],
                                         lhsT=kT[64:128, kc, c * P:(c + 1) * P],
                                         rhs=qT[64:128, pair, :],
                                         start=True, stop=True,
                                         tile_position=(64, 0))
                        eab = epool.tile([P, 2, SQ], BF, tag="exp", name="eab")
                        nc.scalar.activation(eab[:], psAB[:], EXPF, scale=EXP_SCALE)
                        eabs[c] = eab
                    if c >= 1:
                        cc_ = c - 1      # attn.V lags one chunk behind exp
                        eab = eabs.pop(cc_)
                        nc.tensor.matmul(psOA[:VW, :SQ],
                                         lhsT=v_aug[:, kva, cc_, :],
                                         rhs=eab[:, 0, :],
                                         start=(cc_ == 0), stop=(cc_ == SC - 1))
                        nc.tensor.matmul(psOB[:VW, :SQ],
                                         lhsT=v_aug[:, kvb, cc_, :],
                                         rhs=eab[:, 1, :],
                                         start=(cc_ == 0), stop=(cc_ == SC - 1))
                    # spread next-pair q-proj and prev-group out-proj through
                    # the chunk loop so the PE never bunches them at the
                    # pair boundary (ACT rides its 1-chunk buffer)
                    if c == 1 and g >= 1:
                        wo_cur[0] = wo_load(g - 1, pi)
                    if c == 5 and 6 <= pair + 2 < NQ // 2:
                        qproj(pair + 2)
                    if c in (9, 11, 13, 15) and g >= 1:
                        out_proj_m(g - 1, pi, wo_cur[0], (c - 9) // 2)
                oT = oT_tiles[g]
                for psO, dst0 in ((psOA, 0), (psOB, 64)):
                    rbc = npool.tile([32, SQ], FP, tag="rbc", name="rbc")
                    nc.vector.reciprocal(rbc[:], psO[HD:VW, :SQ])
                    nc.vector.tensor_mul(oT[dst0:dst0 + 32, pi, :],
                                         psO[0:32, :SQ], rbc[:])
                    nc.vector.tensor_mul(oT[dst0 + 32:dst0 + 64, pi, :],
                                         psO[32:64, :SQ], rbc[:])

            if g == 3:
                for nf in range(4):
                    wo_nf = wo_load(3, nf)
                    for m in range(RQ):
                        out_proj_m(3, nf, wo_nf, m)

    nc.compile()
    return nc


_NC = None


def _get_nc():
    global _NC
    if _NC is None:
        _NC = build()
    return _NC


def _host_prep(inputs):
    """Permute wq/wk to half-rotated layout, swizzle all weights into the
    on-chip layouts (so device DMAs are linear), build CC/SS tables, slice
    per-core shards."""
    x = np.asarray(inputs["x"], np.float32)
    cos = np.asarray(inputs["cos"], np.float32)
    sin = np.asarray(inputs["sin"], np.float32)
    wq = np.asarray(inputs["wq"], np.float32)
    wk = np.asarray(inputs["wk"], np.float32)
    wv = np.asarray(inputs["wv"], np.float32)
    wo = np.asarray(inputs["wo"], np.float32)

    def perm_cols(w, nheads):
        idx = np.empty(nheads * HD, np.int64)
        for h in range(nheads):
            idx[h * HD:h * HD + 32] = h * HD + 2 * np.arange(32)
            idx[h * HD + 32:(h + 1) * HD] = h * HD + 2 * np.arange(32) + 1
        return np.ascontiguousarray(w[:, idx])

    wq_p = perm_cols(wq, NQ)
    wk_p = perm_cols(wk, NKV)
    # device layouts
    BFH = ml_dtypes.bfloat16
    wq_dev = np.ascontiguousarray(
        wq_p.reshape(DC, P, DC, P).transpose(2, 1, 0, 3)).astype(BFH)
    wk_dev = np.ascontiguousarray(
        wk_p.reshape(DC, P, KF).transpose(1, 0, 2)).astype(BFH)
    wv_dev = np.ascontiguousarray(
        wv.reshape(DC, P, KF).transpose(1, 0, 2)).astype(BFH)
    wo_dev = np.ascontiguousarray(
        wo.reshape(RQ, RQ, P, RQ, 512).transpose(0, 3, 2, 1, 4)).astype(BFH)

    cosT = np.ascontiguousarray(cos.T)            # [32, S]
    sinT = np.ascontiguousarray(sin.T)
    CC = np.tile(cosT, (4, 1))                    # [128, S]
    SS = np.concatenate([-sinT, sinT, -sinT, sinT], 0)

    in_maps = []
    for c in range(NCORES):
        b, q = c // GPB, c % GPB
        sl = slice(q * SQ, (q + 1) * SQ)
        x_dev = np.ascontiguousarray(
            x[b, sl, :].T.reshape(DC, P, SQ).transpose(1, 0, 2)).astype(
                ml_dtypes.bfloat16)
        in_maps.append({
            "x": x_dev,
            "wq": wq_dev, "wk": wk_dev, "wv": wv_dev, "wo": wo_dev,
            "cc": np.ascontiguousarray(CC[:, sl]),
            "ss": np.ascontiguousarray(SS[:, sl]),
        })
    return in_maps


def kernel(**inputs):
    nc = _get_nc()
    in_maps = _host_prep(inputs)
    res = run_bass_kernel_spmd(nc, in_maps, core_ids=list(range(NCORES)))
    out = np.empty((B, S, D), np.float32)
    for c in range(NCORES):
        b, q = c // GPB, c % GPB
        out[b, q * SQ:(q + 1) * SQ, :] = res.results[c]["out"]
    return out



# revision 7
# speedup vs baseline: 1.0111x; 1.0111x over previous
"""Distributed GQA attention (llama-style RoPE) for one TRN2 chip (8 NeuronCores).

Sharding: core c handles batch b=c//4 and sequence-quarter q=c%4 (512 q-rows).
Each core projects Q for its own rows (all 32 heads), projects K/V for its own
512 positions, AllGathers K/V within its 4-core batch group, runs attention for
its rows, and applies the output projection. Output rows are disjoint across
cores, so no all-reduce is needed; the host concatenates.

On-chip dataflow (per core):
  xT   = x.T (pre-transposed on host)                [d, rows]   bf16
  kT   = wk.T @ xT   -> RoPE -> bf16 -> AllGather    [feat, skv]
  v    = xT.T @ wv   -> bf16 -> AllGather            [skv, feat]
  per head pair (qproj + prev-group out-proj interleaved in the chunk loop):
    qT   = wq.T @ xT -> RoPE -> bf16                 [feat, sq]
    sT   = kT_h.T @ qT_h  (row-packed pairs)         [skv, sq]  psum f32
    e    = exp(sT/8) on ScalarE -> bf16
    oTr  = e_blk.T @ [v|1]  accum over skv chunks    [sq, 65]   psum f32
           (transposed AV: out rows = q positions, col 64 = softmax denom;
            costs 65 moving rows/chunk vs 2x512 for the straight form)
    o_n  = oTr[:, :64] * recip(oTr[:, 64])  -> bf16  [sq, feat]
    oT   = PE-transpose(o_n)                         [feat, sq]
  out  = oT.T @ wo, 4 head-group partials accumulated in SBUF via DVE
         (GPSIMD cannot access PSUM on TRN2), streamed to DRAM.

PSUM budget (8 banks): scores/exp 2x2, qproj/outproj 2x1, AV accum 2x1.
The AV banks hold 4 accumulation regions each ([128,65] f32); only the first
region per bank issues start=True (flags the whole 2KB bank pending-zero),
later regions ride the bank flags with start=False - in-order PE issue makes
this safe.  Per-pair transposes borrow a scores-pool slot (bf16, half bank).

All weights are pre-swizzled on the HOST into the exact SBUF layouts so every
DMA is a fully-linear copy. RoPE uses the half-rotated layout: wq/wk columns
are permuted on the host so each head's features are [evens(32) | odds(32)];
cos/sin tables are shipped pre-transposed/tiled as CC/SS [128, sq].
"""
import sys

sys.path.insert(0, "/opt/trn_rl_repo")

import numpy as np
import ml_dtypes
from contextlib import ExitStack

import concourse.bass as bass
import concourse.mybir as mybir
import concourse.tile as tile
from concourse import bacc
from concourse.bass_utils import run_bass_kernel_spmd
from concourse.masks import make_identity

B, S, D = 2, 2048, 2048
NQ, NKV, HD = 32, 8, 64
NCORES = 8
GPB = 4                 # cores per batch group
SQ = S // GPB           # 512 q-rows per core
P = 128
DC = D // P             # 16 contraction chunks
KF = NKV * HD           # 512 kv feature dim
KFC = KF // P           # 4 kv feature chunks
SC = S // P             # 16 skv chunks
RQ = SQ // P            # 4 q-row blocks
AW = HD + 1             # AV width: 64 v cols + 1 ones col (softmax denom)

FP = mybir.dt.float32
BF = mybir.dt.bfloat16
EXPF = mybir.ActivationFunctionType.Exp
EXP_SCALE = 1.0 / 8.0   # 1/sqrt(HD)


def build(solo=False):
    nc = bacc.Bacc("TRN2", target_bir_lowering=False, debug=False,
                   num_devices=1 if solo else NCORES)

    x_e = nc.dram_tensor("x", [P, DC, SQ], BF, kind="ExternalInput").ap()
    wq_e = nc.dram_tensor("wq", [DC, P, DC, P], BF, kind="ExternalInput").ap()
    wk_e = nc.dram_tensor("wk", [P, DC, KF], BF, kind="ExternalInput").ap()
    wv_e = nc.dram_tensor("wv", [P, DC, KF], BF, kind="ExternalInput").ap()
    wo_e = nc.dram_tensor("wo", [RQ, RQ, P, RQ, 512], BF, kind="ExternalInput").ap()
    cc_e = nc.dram_tensor("cc", [P, SQ], FP, kind="ExternalInput").ap()
    ss_e = nc.dram_tensor("ss", [P, SQ], FP, kind="ExternalInput").ap()
    out_e = nc.dram_tensor("out", [SQ, D], FP, kind="ExternalOutput").ap()

    groups = [[0, 1, 2, 3], [4, 5, 6, 7]]

    with tile.TileContext(nc) as tc, ExitStack() as ctx:
        sb = ctx.enter_context(tc.tile_pool(name="sb", bufs=1))
        rp = ctx.enter_context(tc.tile_pool(name="rp", bufs=3))
        epool = ctx.enter_context(tc.tile_pool(name="epool", bufs=6))
        npool = ctx.enter_context(tc.tile_pool(name="npool", bufs=3))
        opool = ctx.enter_context(tc.tile_pool(name="opool", bufs=4))
        otp = ctx.enter_context(tc.tile_pool(name="otp", bufs=2))
        early = ctx.enter_context(tc.tile_pool(name="early", bufs=1))
        wqp = ctx.enter_context(tc.tile_pool(name="wqp", bufs=3))
        dram = ctx.enter_context(tc.tile_pool(name="dram", bufs=1, space="DRAM"))
        pp = ctx.enter_context(tc.tile_pool(name="pp", bufs=2, space="PSUM"))
        psc = ctx.enter_context(tc.tile_pool(name="psc", bufs=2, space="PSUM"))
        av = ctx.enter_context(tc.tile_pool(name="av", bufs=2, space="PSUM"))

        # ---- constants ----
        cc_sb = sb.tile([P, SQ], FP)
        ss_sb = sb.tile([P, SQ], FP)
        id_sb = sb.tile([P, P], BF)

        def rope_chunk(ps, dst):
            """dst = RoPE(ps) in half-rotated layout; ps [128,SQ] psum f32."""
            t0 = rp.tile([P, SQ], FP, tag="t0")
            t1 = rp.tile([P, SQ], FP, tag="t1")
            nc.vector.tensor_mul(t0[:], ps[:], cc_sb[:])
            for blk in range(4):
                o0, i0 = blk * 32, (blk ^ 1) * 32
                nc.vector.tensor_mul(t1[o0:o0 + 32, :], ps[i0:i0 + 32, :],
                                     ss_sb[o0:o0 + 32, :])
            nc.vector.tensor_add(dst, t0[:], t1[:])

        qT = sb.tile([P, DC, SQ], BF)
        kag_in = dram.tile([KF, SQ], BF)
        kag_out = dram.tile([GPB * KF, SQ], BF)
        vag_in = dram.tile([SQ, KF], BF)
        vag_out = dram.tile([S, KF], BF)

        # ---- pure input loads first (sync queue stays unblocked);
        #      x arrives pre-transposed from the host as xT [p, dc, s];
        #      wk rides the scalar-engine DMA queue so it doesn't delay
        #      the loads qproj(0) needs ----
        xT = early.tile([P, DC, SQ], BF, tag="xT", name="xT")
        nc.sync.dma_start(xT[:], x_e)
        wq_tiles = {}
        for pair in (0, 1):
            w = wqp.tile([P, DC, P], BF, tag="wq", name=f"wq_{pair}")
            nc.sync.dma_start(w[:], wq_e[pair])
            wq_tiles[pair] = w
        nc.sync.dma_start(cc_sb[:], cc_e)
        nc.sync.dma_start(ss_sb[:], ss_e)
        wkq = early.tile([P, DC, KF], BF, tag="wkq", name="wkq")
        nc.scalar.dma_start(wkq[:], wk_e)
        make_identity(nc, id_sb)

        def qproj_mm(pair, dcs):
            """Matmul part of the Q projection for `pair`, chunks `dcs`."""
            if pair in wq_tiles:
                wq_sb = wq_tiles.pop(pair)
            else:
                wq_sb = wqp.tile([P, DC, P], BF, tag="wq", name=f"wq_{pair}")
                nc.sync.dma_start(wq_sb[:], wq_e[pair])
            qps = pp.tile([P, 512], FP, tag="pp", name="qps")
            qproj_ps[pair] = (wq_sb, qps)
            qproj_more(pair, dcs)

        def qproj_more(pair, dcs):
            wq_sb, qps = qproj_ps[pair]
            for dc in dcs:
                nc.tensor.matmul(qps[:, :SQ], lhsT=wq_sb[:, dc, :],
                                 rhs=xT[:, dc, :],
                                 start=(dc == 0), stop=(dc == DC - 1))
            if dcs[-1] == DC - 1:
                rope_chunk(qps[:, :SQ], qT[:, pair, :])
                del qproj_ps[pair]

        qproj_ps = {}

        def qproj(pair):
            qproj_mm(pair, list(range(DC)))

        # q-projection for the first two pairs fills the wk-load bubble
        qproj(0)
        qproj(1)

        # ---- K projection + RoPE -> AllGather ----
        kT_own = sb.tile([P, KFC, SQ], BF, tag="own4", name="kT_own")
        for fc in range(KFC):
            ps = pp.tile([P, 512], FP, tag="pp", name="kps")
            for dc in range(DC):
                nc.tensor.matmul(ps[:, :SQ],
                                 lhsT=wkq[:, dc, fc * P:(fc + 1) * P],
                                 rhs=xT[:, dc, :],
                                 start=(dc == 0), stop=(dc == DC - 1))
            rope_chunk(ps[:, :SQ], kT_own[:, fc, :])
        nc.sync.dma_start(kag_in[:].rearrange("(c p) s -> p c s", p=P),
                          kT_own[:])
        if solo:
            for r in range(GPB):
                nc.sync.dma_start(kag_out[r * KF:(r + 1) * KF, :], kag_in[:])
        else:
            nc.gpsimd.collective_compute(
                "AllGather", mybir.AluOpType.bypass, replica_groups=groups,
                ins=[kag_in[:]], outs=[kag_out[:]])

        # q-projection for pairs 2/3 fills the wv-load bubble
        qproj(2)
        qproj(3)

        # ---- V projection -> AllGather ----
        # wv reuses the x slot (x is dead once transposes finished)
        wv_sb = early.tile([P, DC, KF], BF, tag="big32", name="wv_sb")
        nc.sync.dma_start(wv_sb[:], wv_e)
        v_own = sb.tile([P, RQ, KF], BF, tag="own4", name="v_own")
        for pc in range(RQ):
            ps = pp.tile([P, 512], FP, tag="pp", name="vps")
            for dc in range(DC):
                nc.tensor.matmul(ps[:, :KF],
                                 lhsT=xT[:, dc, pc * P:(pc + 1) * P],
                                 rhs=wv_sb[:, dc, :],
                                 start=(dc == 0), stop=(dc == DC - 1))
            nc.vector.tensor_copy(v_own[:, pc, :], ps[:, :KF])
        nc.sync.dma_start(vag_in[:].rearrange("(c p) f -> p c f", p=P),
                          v_own[:])
        if solo:
            for r in range(GPB):
                nc.sync.dma_start(vag_out[r * SQ:(r + 1) * SQ, :], vag_in[:])
        else:
            nc.gpsimd.collective_compute(
                "AllGather", mybir.AluOpType.bypass, replica_groups=groups,
                ins=[vag_in[:]], outs=[vag_out[:]])

        # q-projection for pairs 4/5 fills the AllGather wait window
        qproj(4)
        qproj(5)

        # ---- land gathered K/V (kT reuses the wk slot) ----
        kT = early.tile([P, KFC, S], BF, tag="wkq", name="kT")
        for fc in range(KFC):
            for r in range(GPB):
                nc.scalar.dma_start(kT[:, fc, r * SQ:(r + 1) * SQ],
                                    kag_out[r * KF + fc * P: r * KF + (fc + 1) * P, :])
        v_aug = early.tile([P, NKV, SC, AW], BF, tag="big32", name="v_aug")
        nc.gpsimd.memset(v_aug[:, :, :, HD:AW], 1.0)
        for c in range(SC):
            nc.sync.dma_start(
                v_aug[:, :, c, 0:HD],
                vag_out[c * P:(c + 1) * P, :].rearrange("p (kv d) -> p kv d", d=HD))

        # ---- per-pair: Q proj + attention; prev group's out-proj interleaved ----
        oT_tiles = {}

        def wo_load(g, nf):
            wo_nf = opool.tile([P, 4, 512], BF, tag="wo", name="wo_nf")
            nc.sync.dma_start(wo_nf[:], wo_e[g, nf])
            return wo_nf

        out_acc = sb.tile([P, RQ, D], FP)

        def out_proj_m(g, nf, wo_nf, m):
            """Emit one [128-row, 512-col] tile of group g's out-projection,
            accumulated in SBUF on the Pool engine; group 3 streams the
            finished slice out."""
            oT = oT_tiles[g]
            ps = pp.tile([P, 512], FP, tag="pp", name="ops")
            for ch in range(4):
                nc.tensor.matmul(ps[:],
                                 lhsT=oT[:, ch, m * P:(m + 1) * P],
                                 rhs=wo_nf[:, ch, :],
                                 start=(ch == 0), stop=(ch == 3))
            acc = out_acc[:, m, nf * 512:(nf + 1) * 512]
            if g == 0:
                nc.vector.tensor_copy(acc, ps[:])
            else:
                nc.vector.tensor_add(acc, acc, ps[:])
            if g == 3:
                nc.gpsimd.dma_start(
                    out_e[m * P:(m + 1) * P, nf * 512:(nf + 1) * 512], acc)

        for g in range(4):                    # 4 groups of 4 pairs
            oT_tiles[g] = otp.tile([P, RQ, SQ], BF, tag="oT", name=f"oT_{g}")
            for pi in range(4):               # pairs within group
                pair = g * 4 + pi
                wo_cur = [None]
                kc = pair % 4                 # kv chunk holding both kv heads
                kva, kvb = 2 * (pair % 4), 2 * (pair % 4) + 1
                do_qp = 6 <= pair + 2 < NQ // 2

                avA = av.tile([P, 4, AW], FP, tag="av", name="avA")
                avB = av.tile([P, 4, AW], FP, tag="av", name="avB")
                eabs = {}
                for c in range(SC + 1):
                    if c < SC:
                        # scores for both heads of the pair into one 2-bank
                        # psum tile; one exp op covers A and B
                        psAB = psc.tile([P, 1024], FP, tag="psc", name="psAB")
                        nc.tensor.matmul(psAB[:, 0:SQ],
                                         lhsT=kT[0:64, kc, c * P:(c + 1) * P],
                                         rhs=qT[0:64, pair, :],
                                         start=True, stop=True,
                                         tile_position=(0, 0))
                        nc.tensor.matmul(psAB[:, SQ:2 * SQ],
                                         lhsT=kT[64:128, kc, c * P:(c + 1) * P],
                                         rhs=qT[64:128, pair, :],
                                         start=True, stop=True,
                                         tile_position=(64, 0))
                        eab = epool.tile([P, 2, SQ], BF, tag="exp", name="eab")
                        nc.scalar.activation(eab[:], psAB[:], EXPF, scale=EXP_SCALE)
                        eabs[c] = eab
                    if c >= 1:
                        cc_ = c - 1      # attn.V lags one chunk behind exp
                        eab = eabs.pop(cc_)
                        # transposed AV: out rows = q positions of one block,
                        # accumulate over kv chunks.  Region 0 of each bank
                        # issues start (flags the whole bank pending-zero);
                        # the rest ride the flags with start=False.
                        for ti, tl in ((0, avA), (1, avB)):
                            for ri in range(4):
                                blk = ti * 2 + ri // 2
                                h = ri % 2
                                nc.tensor.matmul(
                                    tl[:, ri, :],
                                    lhsT=eab[:, h, blk * P:(blk + 1) * P],
                                    rhs=v_aug[:, kva + h, cc_, :],
                                    start=(cc_ == 0 and ri == 0),
                                    stop=(cc_ == SC - 1),
                                    skip_group_check=True)
                    # spread next-pair q-proj and prev-group out-proj through
                    # the chunk loop so the PE never bunches them at the
                    # pair boundary (ACT rides its 1-chunk buffer)
                    if do_qp and 1 <= c <= 4:
                        # 4 dc-chunks per score chunk: keeps ACT fed
                        dcs = list(range((c - 1) * 4, c * 4))
                        if c == 1:
                            qproj_mm(pair + 2, dcs)
                        else:
                            qproj_more(pair + 2, dcs)
                    if c == 5 and g >= 1:
                        wo_cur[0] = wo_load(g - 1, pi)
                    if c in (7, 9, 11, 13) and g >= 1:
                        out_proj_m(g - 1, pi, wo_cur[0], (c - 7) // 2)
                # ---- normalize (per-partition scalar), PE-transpose to oT ----
                oT = oT_tiles[g]
                o_n = npool.tile([P, RQ, P], BF, tag="onorm", name="o_n")
                rc = npool.tile([P, 8], FP, tag="rc", name="rc")
                nc.vector.reciprocal(rc[:, 0:4], avA[:, :, HD:AW])
                nc.vector.reciprocal(rc[:, 4:8], avB[:, :, HD:AW])
                for blk in range(RQ):
                    tl = avA if blk < 2 else avB
                    for h in range(2):
                        ri = (blk % 2) * 2 + h
                        base = 0 if blk < 2 else 4
                        nc.vector.tensor_scalar_mul(
                            o_n[:, blk, h * HD:(h + 1) * HD],
                            tl[:, ri, 0:HD],
                            rc[:, base + ri:base + ri + 1])
                tp = psc.tile([P, 1024], BF, tag="psc", name="tp")
                for blk in range(RQ):
                    nc.tensor.transpose(tp[:, blk * P:(blk + 1) * P],
                                        o_n[:, blk, :], id_sb[:])
                nc.vector.tensor_copy(oT[:, pi, :], tp[:, 0:SQ])

            if g == 3:
                for nf in range(4):
                    wo_nf = wo_load(3, nf)
                    for m in range(RQ):
                        out_proj_m(3, nf, wo_nf, m)

    nc.compile()
    return nc


_NC = None


def _get_nc():
    global _NC
    if _NC is None:
        _NC = build()
    return _NC


def _host_prep(inputs):
    """Permute wq/wk to half-rotated layout, swizzle all weights into the
    on-chip layouts (so device DMAs are linear), build CC/SS tables, slice
    per-core shards."""
    x = np.asarray(inputs["x"], np.float32)
    cos = np.asarray(inputs["cos"], np.float32)
    sin = np.asarray(inputs["sin"], np.float32)
    wq = np.asarray(inputs["wq"], np.float32)
    wk = np.asarray(inputs["wk"], np.float32)
    wv = np.asarray(inputs["wv"], np.float32)
    wo = np.asarray(inputs["wo"], np.float32)

    def perm_cols(w, nheads):
        idx = np.empty(nheads * HD, np.int64)
        for h in range(nheads):
            idx[h * HD:h * HD + 32] = h * HD + 2 * np.arange(32)
            idx[h * HD + 32:(h + 1) * HD] = h * HD + 2 * np.arange(32) + 1
        return np.ascontiguousarray(w[:, idx])

    wq_p = perm_cols(wq, NQ)
    wk_p = perm_cols(wk, NKV)
    # device layouts
    BFH = ml_dtypes.bfloat16
    wq_dev = np.ascontiguousarray(
        wq_p.reshape(DC, P, DC, P).transpose(2, 1, 0, 3)).astype(BFH)
    wk_dev = np.ascontiguousarray(
        wk_p.reshape(DC, P, KF).transpose(1, 0, 2)).astype(BFH)
    wv_dev = np.ascontiguousarray(
        wv.reshape(DC, P, KF).transpose(1, 0, 2)).astype(BFH)
    wo_dev = np.ascontiguousarray(
        wo.reshape(RQ, RQ, P, RQ, 512).transpose(0, 3, 2, 1, 4)).astype(BFH)

    cosT = np.ascontiguousarray(cos.T)            # [32, S]
    sinT = np.ascontiguousarray(sin.T)
    CC = np.tile(cosT, (4, 1))                    # [128, S]
    SS = np.concatenate([-sinT, sinT, -sinT, sinT], 0)

    in_maps = []
    for c in range(NCORES):
        b, q = c // GPB, c % GPB
        sl = slice(q * SQ, (q + 1) * SQ)
        x_dev = np.ascontiguousarray(
            x[b, sl, :].T.reshape(DC, P, SQ).transpose(1, 0, 2)).astype(
                ml_dtypes.bfloat16)
        in_maps.append({
            "x": x_dev,
            "wq": wq_dev, "wk": wk_dev, "wv": wv_dev, "wo": wo_dev,
            "cc": np.ascontiguousarray(CC[:, sl]),
            "ss": np.ascontiguousarray(SS[:, sl]),
        })
    return in_maps


def kernel(**inputs):
    nc = _get_nc()
    in_maps = _host_prep(inputs)
    res = run_bass_kernel_spmd(nc, in_maps, core_ids=list(range(NCORES)))
    out = np.empty((B, S, D), np.float32)
    for c in range(NCORES):
        b, q = c // GPB, c % GPB
        out[b, q * SQ:(q + 1) * SQ, :] = res.results[c]["out"]
    return out


# revision 12
# speedup vs baseline: 1.1371x; 1.1246x over previous
"""Distributed GQA attention (llama-style RoPE) for one TRN2 chip (8 NeuronCores).

Sharding: core c handles batch b=c//4 and sequence-quarter q=c%4 (512 q-rows).
Each core projects Q for its own rows (all 32 heads), projects K/V for its own
512 positions, AllGathers K/V within its 4-core batch group, runs attention for
its rows, and applies the output projection. Output rows are disjoint across
cores, so no all-reduce is needed; the host concatenates.

On-chip dataflow (per core):
  xT   = x.T (pre-transposed on host, landed in 4 column chunks)  [d, rows]
  kT   = wk.T @ xT  -> RoPE -> bf16 -> AllGather (one per feature chunk,
         dispatched as soon as that chunk's rope is done) -> [feat, skv].
         kproj runs FIRST so the ~267us exp stream can start early.
  v    = xT.T @ wv  -> bf16 (PSUM->SBUF copy on the idle ACT engine)
         -> AllGather -> [skv, feat]
  per head pair:
    qT   = wq.T @ xT -> RoPE -> bf16                 [feat, sq]
    sT   = kT_h.T @ qT_h  (row-packed pairs)         [skv, sq]  psum f32
    e    = exp(sT/8) on ScalarE -> bf16
    oTr  = e_blk.T @ [v|1]  accum over skv chunks    [sq, 65]   psum f32
           (transposed AV: out rows = q positions, col 64 = softmax denom;
            costs 65 moving rows/chunk vs 2x512 for the straight form)
    o_n  = oTr[:, :64] * recip(oTr[:, 64])  -> bf16  [sq, feat]
    oT   = PE-transpose(o_n)                         [feat, sq]
  out  = oT.T @ wo, 4 head-group partials accumulated in SBUF via DVE
         (GPSIMD cannot access PSUM on TRN2), streamed to DRAM on two queues.

Schedule: every engine queue executes IN ORDER, so instruction emission order
is the schedule.  Per pair (17-step chunk loop, attn.V lagging 2 chunks):
  c0-c3 : next q-projection, 4 dc-chunks per step (pairs 0-3 carry a second
          q-projection at c6-c9 - they have no out-proj to interleave)
  c0/c3/c5: previous pair's normalize / PE-transpose / copy-to-oT, placed so
          the DVE normalize is certainly done before the PE transposes issue
  c7,9,11,13: out-proj quarters of the previous group (+ c5: wo prefetch,
          c10: wq prefetch for the next pair's q-projection)
PSUM (8 banks): scores/exp 2 bufs x 2 banks; two explicitly-addressed 1-bank
slots ppA (q-proj, even out-proj tiles) and ppB (transposes, odd out-proj
tiles, second q-projection) - explicit tags, not round-robin, so no request
ever queues behind a slot whose consumer hasn't run yet; AV accum 2 x 1 bank.
The AV banks hold 4 accumulation regions each ([128,65] f32); only the first
region per bank issues start=True (flags the whole 2KB bank pending-zero),
later regions ride the bank flags with start=False - in-order PE issue makes
this safe.

All weights are pre-swizzled on the HOST into the exact SBUF layouts so every
DMA is a fully-linear copy. RoPE uses the half-rotated layout: wq/wk columns
are permuted on the host so each head's features are [evens(32) | odds(32)];
cos/sin tables are shipped pre-transposed/tiled as CC/SS [128, sq].
"""
import sys

sys.path.insert(0, "/opt/trn_rl_repo")

import numpy as np
import ml_dtypes
from contextlib import ExitStack

import concourse.bass as bass
import concourse.mybir as mybir
import concourse.tile as tile
from concourse import bacc
from concourse.bass_utils import run_bass_kernel_spmd
from concourse.masks import make_identity

B, S, D = 2, 2048, 2048
NQ, NKV, HD = 32, 8, 64
NCORES = 8
GPB = 4                 # cores per batch group
SQ = S // GPB           # 512 q-rows per core
P = 128
DC = D // P             # 16 contraction chunks
KF = NKV * HD           # 512 kv feature dim
KFC = KF // P           # 4 kv feature chunks
SC = S // P             # 16 skv chunks
RQ = SQ // P            # 4 q-row blocks
AW = HD + 1             # AV width: 64 v cols + 1 ones col (softmax denom)

FP = mybir.dt.float32
BF = mybir.dt.bfloat16
EXPF = mybir.ActivationFunctionType.Exp
EXP_SCALE = 1.0 / 8.0   # 1/sqrt(HD)


def build(solo=False):
    nc = bacc.Bacc("TRN2", target_bir_lowering=False, debug=False,
                   num_devices=1 if solo else NCORES)

    x_e = nc.dram_tensor("x", [P, DC, SQ], BF, kind="ExternalInput").ap()
    wq_e = nc.dram_tensor("wq", [DC, P, DC, P], BF, kind="ExternalInput").ap()
    wk_e = nc.dram_tensor("wk", [P, DC, KF], BF, kind="ExternalInput").ap()
    wv_e = nc.dram_tensor("wv", [P, DC, KF], BF, kind="ExternalInput").ap()
    wo_e = nc.dram_tensor("wo", [RQ, RQ, P, RQ, 512], BF, kind="ExternalInput").ap()
    cc_e = nc.dram_tensor("cc", [P, SQ], FP, kind="ExternalInput").ap()
    ss_e = nc.dram_tensor("ss", [P, SQ], FP, kind="ExternalInput").ap()
    out_e = nc.dram_tensor("out", [SQ, D], FP, kind="ExternalOutput").ap()

    groups = [[0, 1, 2, 3], [4, 5, 6, 7]]

    with tile.TileContext(nc) as tc, ExitStack() as ctx:
        sb = ctx.enter_context(tc.tile_pool(name="sb", bufs=1))
        rp = ctx.enter_context(tc.tile_pool(name="rp", bufs=3))
        epool = ctx.enter_context(tc.tile_pool(name="epool", bufs=6))
        npool = ctx.enter_context(tc.tile_pool(name="npool", bufs=3))
        opool = ctx.enter_context(tc.tile_pool(name="opool", bufs=4))
        otp = ctx.enter_context(tc.tile_pool(name="otp", bufs=2))
        early = ctx.enter_context(tc.tile_pool(name="early", bufs=1))
        wqp = ctx.enter_context(tc.tile_pool(name="wqp", bufs=4))
        dram = ctx.enter_context(tc.tile_pool(name="dram", bufs=1, space="DRAM"))
        pp = ctx.enter_context(tc.tile_pool(name="pp", bufs=1, space="PSUM"))
        psc = ctx.enter_context(tc.tile_pool(name="psc", bufs=2, space="PSUM"))
        av = ctx.enter_context(tc.tile_pool(name="av", bufs=2, space="PSUM"))

        # ---- constants ----
        cc_sb = sb.tile([P, SQ], FP)
        ss_sb = sb.tile([P, SQ], FP)
        id_sb = sb.tile([P, P], BF)

        def rope_chunk(ps, dst):
            """dst = RoPE(ps) in half-rotated layout; ps [128,SQ] psum f32."""
            t0 = rp.tile([P, SQ], FP, tag="t0")
            t1 = rp.tile([P, SQ], FP, tag="t1")
            nc.vector.tensor_mul(t0[:], ps[:], cc_sb[:])
            for blk in range(4):
                o0, i0 = blk * 32, (blk ^ 1) * 32
                nc.vector.tensor_mul(t1[o0:o0 + 32, :], ps[i0:i0 + 32, :],
                                     ss_sb[o0:o0 + 32, :])
            nc.vector.tensor_add(dst, t0[:], t1[:])

        qT = sb.tile([P, DC, SQ], BF)
        kag_in = dram.tile([KFC, P, SQ], BF)
        kag_out = dram.tile([KFC, GPB, P, SQ], BF)
        vag_in = dram.tile([SQ, KF], BF)
        vag_out = dram.tile([S, KF], BF)

        # ---- input loads spread over three DMA queues; x lands in 4
        #      column-chunks so kproj can start contracting early ----
        xT = early.tile([P, DC, SQ], BF, tag="xT", name="xT")
        for qc in range(4):
            nc.sync.dma_start(xT[:, qc * 4:(qc + 1) * 4, :],
                              x_e[:, qc * 4:(qc + 1) * 4, :])
        nc.sync.dma_start(cc_sb[:], cc_e)
        nc.sync.dma_start(ss_sb[:], ss_e)
        wkq = early.tile([P, DC, KF], BF, tag="wkq", name="wkq")
        nc.scalar.dma_start(wkq[:], wk_e)
        wq_tiles = {}

        def wq_prefetch(pair):
            w = wqp.tile([P, DC, P], BF, tag="wq", name=f"wq_{pair}")
            nc.sync.dma_start(w[:], wq_e[pair])
            wq_tiles[pair] = w

        wq_prefetch(0)
        wq_prefetch(1)
        wv_sb = early.tile([P, DC, KF], BF, tag="big32", name="wv_sb")
        nc.gpsimd.dma_start(wv_sb[:], wv_e)
        make_identity(nc, id_sb)

        def pslot(slot, dtype=FP, shape=(P, 512), name="ps"):
            return pp.tile(list(shape), dtype, tag=f"pp{slot}", name=name)

        # ---- K projection + RoPE first; AllGather per feature chunk so the
        #      gather pipeline overlaps the remaining ropes ----
        kT_own = sb.tile([P, KFC, SQ], BF, tag="own4", name="kT_own")
        for fc in range(KFC):
            ps = pslot(fc % 2, name="kps")
            for dc in range(DC):
                nc.tensor.matmul(ps[:, :SQ],
                                 lhsT=wkq[:, dc, fc * P:(fc + 1) * P],
                                 rhs=xT[:, dc, :],
                                 start=(dc == 0), stop=(dc == DC - 1))
            rope_chunk(ps[:, :SQ], kT_own[:, fc, :])
            nc.sync.dma_start(kag_in[fc], kT_own[:, fc, :])
            if solo:
                for r in range(GPB):
                    nc.sync.dma_start(kag_out[fc, r], kag_in[fc])
            else:
                nc.gpsimd.collective_compute(
                    "AllGather", mybir.AluOpType.bypass,
                    replica_groups=groups,
                    ins=[kag_in[fc]], outs=[kag_out[fc]])

        # ---- V projection (PSUM copies on ACT: DVE is busy with k-ropes)
        #      -> AllGather ----
        v_own = sb.tile([P, RQ, KF], BF, tag="own4", name="v_own")
        for pc in range(RQ):
            ps = pslot(pc % 2, name="vps")
            for dc in range(DC):
                nc.tensor.matmul(ps[:, :KF],
                                 lhsT=xT[:, dc, pc * P:(pc + 1) * P],
                                 rhs=wv_sb[:, dc, :],
                                 start=(dc == 0), stop=(dc == DC - 1))
            nc.scalar.copy(v_own[:, pc, :], ps[:, :KF])
        nc.sync.dma_start(vag_in[:].rearrange("(c p) f -> p c f", p=P),
                          v_own[:])
        if solo:
            for r in range(GPB):
                nc.sync.dma_start(vag_out[r * SQ:(r + 1) * SQ, :], vag_in[:])
        else:
            nc.gpsimd.collective_compute(
                "AllGather", mybir.AluOpType.bypass, replica_groups=groups,
                ins=[vag_in[:]], outs=[vag_out[:]])

        qproj_ps = {}

        def qproj_mm(pair, dcs, qps):
            """Emit dc-chunk matmuls of the Q projection for `pair`; rope and
            release the psum slot after the last chunk."""
            if qps is not None:
                qproj_ps[pair] = (wq_tiles.pop(pair), qps)
            wq_sb, qps = qproj_ps[pair]
            for dc in dcs:
                nc.tensor.matmul(qps[:, :SQ], lhsT=wq_sb[:, dc, :],
                                 rhs=xT[:, dc, :],
                                 start=(dc == 0), stop=(dc == DC - 1))
            if dcs[-1] == DC - 1:
                rope_chunk(qps[:, :SQ], qT[:, pair, :])
                del qproj_ps[pair]

        # prologue q-projections for pairs 0/1 ride the idle scores pool;
        # pair 0 consumes wq tiles 2/3 in its own chunk loop - prefetch here
        qproj_mm(0, list(range(DC)),
                 psc.tile([P, 1024], FP, tag="psc", name="q0ps"))
        wq_prefetch(2)
        qproj_mm(1, list(range(DC)),
                 psc.tile([P, 1024], FP, tag="psc", name="q1ps"))
        wq_prefetch(3)

        # ---- land gathered K/V (kT reuses the wk slot) ----
        kT = early.tile([P, KFC, S], BF, tag="wkq", name="kT")
        for fc in range(KFC):
            for r in range(GPB):
                nc.scalar.dma_start(kT[:, fc, r * SQ:(r + 1) * SQ],
                                    kag_out[fc, r])
        v_aug = early.tile([P, NKV, SC, AW], BF, tag="big32", name="v_aug")
        nc.gpsimd.memset(v_aug[:, :, :, HD:AW], 1.0)
        for c in range(SC):
            nc.sync.dma_start(
                v_aug[:, :, c, 0:HD],
                vag_out[c * P:(c + 1) * P, :].rearrange("p (kv d) -> p kv d", d=HD))

        # ---- per-pair attention loop ----
        oT_tiles = {}

        def wo_load(g, nf):
            wo_nf = opool.tile([P, 4, 512], BF, tag="wo", name="wo_nf")
            nc.sync.dma_start(wo_nf[:], wo_e[g, nf])
            return wo_nf

        out_acc = sb.tile([P, RQ, D], FP)

        def out_proj_m(g, nf, wo_nf, m, slot):
            """One [128-row, 512-col] tile of group g's out-projection,
            accumulated in SBUF; group 3 streams the finished slice out on
            alternating DMA queues."""
            oT = oT_tiles[g]
            ps = pslot(slot, name="ops")
            for ch in range(4):
                nc.tensor.matmul(ps[:],
                                 lhsT=oT[:, ch, m * P:(m + 1) * P],
                                 rhs=wo_nf[:, ch, :],
                                 start=(ch == 0), stop=(ch == 3))
            acc = out_acc[:, m, nf * 512:(nf + 1) * 512]
            if g == 0:
                nc.vector.tensor_copy(acc, ps[:])
            else:
                nc.vector.tensor_add(acc, acc, ps[:])
            if g == 3:
                eng = nc.gpsimd if (nf + m) % 2 == 0 else nc.scalar
                eng.dma_start(
                    out_e[m * P:(m + 1) * P, nf * 512:(nf + 1) * 512], acc)

        fin = {}                # previous pair's normalize/transpose state

        def finish_stage(stage):
            """stage 0: normalize (DVE); 1: PE-transpose; 2: copy to oT."""
            if not fin:
                return
            if stage == 0:
                favA, favB = fin["avA"], fin["avB"]
                o_n = npool.tile([P, RQ, P], BF, tag="onorm", name="o_n")
                rc = npool.tile([P, 8], FP, tag="rc", name="rc")
                nc.vector.reciprocal(rc[:, 0:4], favA[:, :, HD:AW])
                nc.vector.reciprocal(rc[:, 4:8], favB[:, :, HD:AW])
                for blk in range(RQ):
                    tl = favA if blk < 2 else favB
                    for h in range(2):
                        ri = (blk % 2) * 2 + h
                        base = 0 if blk < 2 else 4
                        nc.vector.tensor_scalar_mul(
                            o_n[:, blk, h * HD:(h + 1) * HD],
                            tl[:, ri, 0:HD],
                            rc[:, base + ri:base + ri + 1])
                fin["o_n"] = o_n
            elif stage == 1:
                tp = pslot(1, BF, (P, SQ), name="tp")
                for blk in range(RQ):
                    nc.tensor.transpose(tp[:, blk * P:(blk + 1) * P],
                                        fin["o_n"][:, blk, :], id_sb[:])
                fin["tp"] = tp
            else:
                fg, fpi = fin["pair"] // 4, fin["pair"] % 4
                nc.vector.tensor_copy(oT_tiles[fg][:, fpi, :], fin["tp"][:, 0:SQ])
                fin.clear()

        # qproj injection plan: pairs 0-3 carry two qprojs each (pairs 2-9,
        # second one on slot ppB), pairs 4-9 one each (pairs 10-15).
        qplan = {}
        for p in range(4):
            qplan[p] = (2 * p + 2, 2 * p + 3)
        for p in range(4, 10):
            qplan[p] = (p + 6,)

        for g in range(4):                    # 4 groups of 4 pairs
            oT_tiles[g] = otp.tile([P, RQ, SQ], BF, tag="oT", name=f"oT_{g}")
            for pi in range(4):               # pairs within group
                pair = g * 4 + pi
                wo_cur = [None]
                kc = pair % 4                 # kv chunk holding both kv heads
                kva = 2 * (pair % 4)
                qph = qplan.get(pair, ())

                avA = av.tile([P, 4, AW], FP, tag="av", name="avA")
                avB = av.tile([P, 4, AW], FP, tag="av", name="avB")
                eabs = {}
                for c in range(SC + 2):
                    if c < SC:
                        # scores for both heads of the pair into one 2-bank
                        # psum tile; one exp op covers A and B
                        psAB = psc.tile([P, 1024], FP, tag="psc", name="psAB")
                        nc.tensor.matmul(psAB[:, 0:SQ],
                                         lhsT=kT[0:64, kc, c * P:(c + 1) * P],
                                         rhs=qT[0:64, pair, :],
                                         start=True, stop=True,
                                         tile_position=(0, 0))
                        nc.tensor.matmul(psAB[:, SQ:2 * SQ],
                                         lhsT=kT[64:128, kc, c * P:(c + 1) * P],
                                         rhs=qT[64:128, pair, :],
                                         start=True, stop=True,
                                         tile_position=(64, 0))
                        eab = epool.tile([P, 2, SQ], BF, tag="exp", name="eab")
                        nc.scalar.activation(eab[:], psAB[:], EXPF, scale=EXP_SCALE)
                        eabs[c] = eab
                    # previous pair's normalize/transpose/copy-out
                    if c == 0:
                        finish_stage(0)
                    elif c == 3:
                        finish_stage(1)
                    elif c == 5:
                        finish_stage(2)
                    if c >= 2:
                        cc_ = c - 2      # attn.V lags two chunks behind exp
                        eab = eabs.pop(cc_)
                        # transposed AV: out rows = q positions of one block,
                        # accumulate over kv chunks.  Region 0 of each bank
                        # issues start (flags the whole bank pending-zero);
                        # the rest ride the flags with start=False.
                        for ti, tl in ((0, avA), (1, avB)):
                            for ri in range(4):
                                blk = ti * 2 + ri // 2
                                h = ri % 2
                                nc.tensor.matmul(
                                    tl[:, ri, :],
                                    lhsT=eab[:, h, blk * P:(blk + 1) * P],
                                    rhs=v_aug[:, kva + h, cc_, :],
                                    start=(cc_ == 0 and ri == 0),
                                    stop=(cc_ == SC - 1),
                                    skip_group_check=True)
                    # q-projections: 4 dc-chunks per score chunk keeps ACT fed
                    if len(qph) >= 1 and 0 <= c <= 3:
                        qproj_mm(qph[0], list(range(c * 4, (c + 1) * 4)),
                                 pslot(0, name="qps") if c == 0 else None)
                    if len(qph) >= 2 and 6 <= c <= 9:
                        qproj_mm(qph[1], list(range((c - 6) * 4, (c - 5) * 4)),
                                 pslot(1, name="qps") if c == 6 else None)
                    if c == 5 and g >= 1:
                        wo_cur[0] = wo_load(g - 1, pi)
                    if c == 10 and pair + 1 in qplan:
                        for qp in qplan[pair + 1]:
                            wq_prefetch(qp)
                    if c in (7, 9, 11, 13) and g >= 1:
                        out_proj_m(g - 1, pi, wo_cur[0], (c - 7) // 2,
                                   slot=(1 if c in (7, 11) else 0))
                fin.update({"pair": pair, "avA": avA, "avB": avB})

            if g == 3:
                for st in range(3):
                    finish_stage(st)
                for nf in range(4):
                    wo_nf = wo_load(3, nf)
                    for m in range(RQ):
                        out_proj_m(3, nf, wo_nf, m, slot=m % 2)

    nc.compile()
    return nc


_NC = None


def _get_nc():
    global _NC
    if _NC is None:
        _NC = build()
    return _NC


def _host_prep(inputs):
    """Permute wq/wk to half-rotated layout, swizzle all weights into the
    on-chip layouts (so device DMAs are linear), build CC/SS tables, slice
    per-core shards."""
    x = np.asarray(inputs["x"], np.float32)
    cos = np.asarray(inputs["cos"], np.float32)
    sin = np.asarray(inputs["sin"], np.float32)
    wq = np.asarray(inputs["wq"], np.float32)
    wk = np.asarray(inputs["wk"], np.float32)
    wv = np.asarray(inputs["wv"], np.float32)
    wo = np.asarray(inputs["wo"], np.float32)

    def perm_cols(w, nheads):
        idx = np.empty(nheads * HD, np.int64)
        for h in range(nheads):
            idx[h * HD:h * HD + 32] = h * HD + 2 * np.arange(32)
            idx[h * HD + 32:(h + 1) * HD] = h * HD + 2 * np.arange(32) + 1
        return np.ascontiguousarray(w[:, idx])

    wq_p = perm_cols(wq, NQ)
    wk_p = perm_cols(wk, NKV)
    # device layouts
    BFH = ml_dtypes.bfloat16
    wq_dev = np.ascontiguousarray(
        wq_p.reshape(DC, P, DC, P).transpose(2, 1, 0, 3)).astype(BFH)
    wk_dev = np.ascontiguousarray(
        wk_p.reshape(DC, P, KF).transpose(1, 0, 2)).astype(BFH)
    wv_dev = np.ascontiguousarray(
        wv.reshape(DC, P, KF).transpose(1, 0, 2)).astype(BFH)
    wo_dev = np.ascontiguousarray(
        wo.reshape(RQ, RQ, P, RQ, 512).transpose(0, 3, 2, 1, 4)).astype(BFH)

    cosT = np.ascontiguousarray(cos.T)            # [32, S]
    sinT = np.ascontiguousarray(sin.T)
    CC = np.tile(cosT, (4, 1))                    # [128, S]
    SS = np.concatenate([-sinT, sinT, -sinT, sinT], 0)

    in_maps = []
    for c in range(NCORES):
        b, q = c // GPB, c % GPB
        sl = slice(q * SQ, (q + 1) * SQ)
        x_dev = np.ascontiguousarray(
            x[b, sl, :].T.reshape(DC, P, SQ).transpose(1, 0, 2)).astype(
                ml_dtypes.bfloat16)
        in_maps.append({
            "x": x_dev,
            "wq": wq_dev, "wk": wk_dev, "wv": wv_dev, "wo": wo_dev,
            "cc": np.ascontiguousarray(CC[:, sl]),
            "ss": np.ascontiguousarray(SS[:, sl]),
        })
    return in_maps


def kernel(**inputs):
    nc = _get_nc()
    in_maps = _host_prep(inputs)
    res = run_bass_kernel_spmd(nc, in_maps, core_ids=list(range(NCORES)))
    out = np.empty((B, S, D), np.float32)
    for c in range(NCORES):
        b, q = c // GPB, c % GPB
        out[b, q * SQ:(q + 1) * SQ, :] = res.results[c]["out"]
    return out


# revision 15
# speedup vs baseline: 1.1664x; 1.0258x over previous
"""Distributed GQA attention (llama-style RoPE) for one TRN2 chip (8 NeuronCores).

Sharding: core c handles batch b=c//4 and sequence-quarter q=c%4 (512 q-rows).
Each core projects Q for its own rows (all 32 heads), projects K/V for its own
512 positions, AllGathers K/V within its 4-core batch group, runs attention for
its rows, and applies the output projection. Output rows are disjoint across
cores, so no all-reduce is needed; the host concatenates.

On-chip dataflow (per core):
  xT   = x.T (pre-transposed on host, landed in 4 column chunks)  [d, rows]
  kT   = wk.T @ xT  -> RoPE -> bf16 -> AllGather (one per feature chunk,
         dispatched as soon as that chunk's rope is done) -> [feat, skv].
         kproj runs FIRST so the ~267us exp stream can start early.
  v    = xT.T @ wv  -> bf16 (PSUM->SBUF copy on the idle ACT engine)
         -> AllGather -> [skv, feat]
  per head pair:
    qT   = wq.T @ xT -> RoPE -> bf16                 [feat, sq]
    sT   = kT_h.T @ qT_h  (row-packed pairs)         [skv, sq]  psum f32
    e    = exp(sT/8) on ScalarE -> bf16
    oTr  = e_blk.T @ [v|1]  accum over skv chunks    [sq, 65]   psum f32
           (transposed AV: out rows = q positions, col 64 = softmax denom;
            costs 65 moving rows/chunk vs 2x512 for the straight form)
    o_n  = oTr[:, :64] * recip(oTr[:, 64])  -> bf16  [sq, feat]
    oT   = PE-transpose(o_n)                         [feat, sq]
  out  = oT.T @ wo, 4 head-group partials accumulated in SBUF via DVE
         (GPSIMD cannot access PSUM on TRN2), streamed to DRAM on two queues.

Schedule: every engine queue executes IN ORDER, so instruction emission order
is the schedule.  Per pair (17-step chunk loop, attn.V lagging 2 chunks):
  c0-c3 : next q-projection, 4 dc-chunks per step (pairs 0-3 carry a second
          q-projection at c6-c9 - they have no out-proj to interleave)
  c0/c3/c5: previous pair's normalize / PE-transpose / copy-to-oT, placed so
          the DVE normalize is certainly done before the PE transposes issue
  c7,9,11,13: out-proj quarters of the previous group (+ c5: wo prefetch,
          c10: wq prefetch for the next pair's q-projection)
PSUM (8 banks): scores/exp 2 bufs x 2 banks; two explicitly-addressed 1-bank
slots ppA (q-proj, even out-proj tiles) and ppB (transposes, odd out-proj
tiles, second q-projection) - explicit tags, not round-robin, so no request
ever queues behind a slot whose consumer hasn't run yet; AV accum 2 x 1 bank.
The AV banks hold 4 accumulation regions each ([128,65] f32); only the first
region per bank issues start=True (flags the whole 2KB bank pending-zero),
later regions ride the bank flags with start=False - in-order PE issue makes
this safe.

All weights are pre-swizzled on the HOST into the exact SBUF layouts so every
DMA is a fully-linear copy. RoPE uses the half-rotated layout: wq/wk columns
are permuted on the host so each head's features are [evens(32) | odds(32)];
cos/sin tables are shipped pre-transposed/tiled as CC/SS [128, sq].
"""
import sys

sys.path.insert(0, "/opt/trn_rl_repo")

import numpy as np
import ml_dtypes
from contextlib import ExitStack

import concourse.bass as bass
import concourse.mybir as mybir
import concourse.tile as tile
from concourse import bacc
from concourse.bass_utils import run_bass_kernel_spmd
from concourse.masks import make_identity

B, S, D = 2, 2048, 2048
NQ, NKV, HD = 32, 8, 64
NCORES = 8
GPB = 4                 # cores per batch group
SQ = S // GPB           # 512 q-rows per core
P = 128
DC = D // P             # 16 contraction chunks
KF = NKV * HD           # 512 kv feature dim
KFC = KF // P           # 4 kv feature chunks
SC = S // P             # 16 skv chunks
RQ = SQ // P            # 4 q-row blocks
AW = HD + 1             # AV width: 64 v cols + 1 ones col (softmax denom)

FP = mybir.dt.float32
BF = mybir.dt.bfloat16
EXPF = mybir.ActivationFunctionType.Exp
EXP_SCALE = 1.0 / 8.0   # 1/sqrt(HD)


def build(solo=False):
    nc = bacc.Bacc("TRN2", target_bir_lowering=False, debug=False,
                   num_devices=1 if solo else NCORES)

    x_e = nc.dram_tensor("x", [P, DC, SQ], BF, kind="ExternalInput").ap()
    wq_e = nc.dram_tensor("wq", [DC, P, DC, P], BF, kind="ExternalInput").ap()
    wk_e = nc.dram_tensor("wk", [P, DC, KF], BF, kind="ExternalInput").ap()
    wv_e = nc.dram_tensor("wv", [P, DC, KF], BF, kind="ExternalInput").ap()
    wo_e = nc.dram_tensor("wo", [RQ, RQ, P, RQ, 512], BF, kind="ExternalInput").ap()
    cc_e = nc.dram_tensor("cc", [P, SQ], FP, kind="ExternalInput").ap()
    ss_e = nc.dram_tensor("ss", [P, SQ], FP, kind="ExternalInput").ap()
    out_e = nc.dram_tensor("out", [SQ, D], FP, kind="ExternalOutput").ap()

    groups = [[0, 1, 2, 3], [4, 5, 6, 7]]

    with tile.TileContext(nc) as tc, ExitStack() as ctx:
        sb = ctx.enter_context(tc.tile_pool(name="sb", bufs=1))
        rp = ctx.enter_context(tc.tile_pool(name="rp", bufs=3))
        epool = ctx.enter_context(tc.tile_pool(name="epool", bufs=6))
        npool = ctx.enter_context(tc.tile_pool(name="npool", bufs=3))
        opool = ctx.enter_context(tc.tile_pool(name="opool", bufs=4))
        otp = ctx.enter_context(tc.tile_pool(name="otp", bufs=2))
        early = ctx.enter_context(tc.tile_pool(name="early", bufs=1))
        wqp = ctx.enter_context(tc.tile_pool(name="wqp", bufs=4))
        dram = ctx.enter_context(tc.tile_pool(name="dram", bufs=1, space="DRAM"))
        pp = ctx.enter_context(tc.tile_pool(name="pp", bufs=1, space="PSUM"))
        psc = ctx.enter_context(tc.tile_pool(name="psc", bufs=2, space="PSUM"))
        av = ctx.enter_context(tc.tile_pool(name="av", bufs=2, space="PSUM"))

        # ---- constants ----
        cc_sb = sb.tile([P, SQ], FP)
        ss_sb = sb.tile([P, SQ], FP)
        id_sb = sb.tile([P, P], BF)

        def rope_chunk(ps, dst):
            """dst = RoPE(ps) in half-rotated layout; ps [128,SQ] psum f32."""
            t0 = rp.tile([P, SQ], FP, tag="t0")
            t1 = rp.tile([P, SQ], FP, tag="t1")
            nc.vector.tensor_mul(t0[:], ps[:], cc_sb[:])
            for blk in range(4):
                o0, i0 = blk * 32, (blk ^ 1) * 32
                nc.vector.tensor_mul(t1[o0:o0 + 32, :], ps[i0:i0 + 32, :],
                                     ss_sb[o0:o0 + 32, :])
            nc.vector.tensor_add(dst, t0[:], t1[:])

        qT = sb.tile([P, DC, SQ], BF)
        kag_in = dram.tile([KFC, P, SQ], BF)
        kag_out = dram.tile([KFC, GPB, P, SQ], BF)
        vag_in = dram.tile([SQ, KF], BF)
        vag_out = dram.tile([S, KF], BF)

        # ---- input loads: the DMA bus is one shared 360GB/s resource in
        #      arrival order, so emit the bytes the pipeline needs first:
        #      x/wk in interleaved quarter-chunks (kproj contracts as they
        #      land), then rope tables, then wq0, then wv ----
        xT = early.tile([P, DC, SQ], BF, tag="xT", name="xT")
        wkq = early.tile([P, DC, KF], BF, tag="wkq", name="wkq")
        for qc in range(4):
            nc.sync.dma_start(xT[:, qc * 4:(qc + 1) * 4, :],
                              x_e[:, qc * 4:(qc + 1) * 4, :])
            nc.scalar.dma_start(wkq[:, qc * 4:(qc + 1) * 4, :],
                                wk_e[:, qc * 4:(qc + 1) * 4, :])
            if qc == 0:
                nc.sync.dma_start(cc_sb[:], cc_e)
                nc.sync.dma_start(ss_sb[:], ss_e)
        wq_tiles = {}

        def wq_prefetch(pair):
            w = wqp.tile([P, DC, P], BF, tag="wq", name=f"wq_{pair}")
            nc.sync.dma_start(w[:], wq_e[pair])
            wq_tiles[pair] = w

        wq_prefetch(0)
        wv_sb = early.tile([P, DC, KF], BF, tag="big32", name="wv_sb")
        nc.sync.dma_start(wv_sb[:], wv_e)
        wq_prefetch(1)
        make_identity(nc, id_sb)
        ones1 = sb.tile([P, 1], BF)
        nc.gpsimd.memset(ones1[:], 1.0)

        def pslot(slot, dtype=FP, shape=(P, 512), name="ps"):
            return pp.tile(list(shape), dtype, tag=f"pp{slot}", name=name)

        qproj_ps = {}

        def qproj_mm(pair, dcs, qps):
            """Emit dc-chunk matmuls of the Q projection for `pair`; rope and
            release the psum slot after the last chunk."""
            if qps is not None:
                qproj_ps[pair] = (wq_tiles.pop(pair), qps)
            wq_sb, qps = qproj_ps[pair]
            for dc in dcs:
                nc.tensor.matmul(qps[:, :SQ], lhsT=wq_sb[:, dc, :],
                                 rhs=xT[:, dc, :],
                                 start=(dc == 0), stop=(dc == DC - 1))
            if dcs[-1] == DC - 1:
                rope_chunk(qps[:, :SQ], qT[:, pair, :])
                del qproj_ps[pair]

        # ---- K projection + RoPE first; AllGather per feature chunk so the
        #      gather pipeline overlaps the remaining ropes.  qproj(0) sits
        #      between the kproj halves: its rope (which gates pair-0 scores)
        #      lands 3rd in the DVE chain, while AllGathers for the late kv
        #      chunks (first needed by pairs 2/3) may finish late ----
        kT_own = sb.tile([P, KFC, SQ], BF, tag="own4", name="kT_own")

        def kproj(fc):
            ps = pslot(fc % 2, name="kps")
            for dc in range(DC):
                nc.tensor.matmul(ps[:, :SQ],
                                 lhsT=wkq[:, dc, fc * P:(fc + 1) * P],
                                 rhs=xT[:, dc, :],
                                 start=(dc == 0), stop=(dc == DC - 1))
            rope_chunk(ps[:, :SQ], kT_own[:, fc, :])
            nc.sync.dma_start(kag_in[fc], kT_own[:, fc, :])
            if solo:
                for r in range(GPB):
                    nc.sync.dma_start(kag_out[fc, r], kag_in[fc])
            else:
                nc.gpsimd.collective_compute(
                    "AllGather", mybir.AluOpType.bypass,
                    replica_groups=groups,
                    ins=[kag_in[fc]], outs=[kag_out[fc]])

        kproj(0)
        kproj(1)
        qproj_mm(0, list(range(DC)),
                 psc.tile([P, 1024], FP, tag="psc", name="q0ps"))
        kproj(2)
        kproj(3)

        # ---- V projection (PSUM copies on ACT: DVE is busy with ropes)
        #      -> AllGather ----
        v_own = sb.tile([P, RQ, KF], BF, tag="own4", name="v_own")
        for pc in range(RQ):
            ps = pslot(pc % 2, name="vps")
            for dc in range(DC):
                nc.tensor.matmul(ps[:, :KF],
                                 lhsT=xT[:, dc, pc * P:(pc + 1) * P],
                                 rhs=wv_sb[:, dc, :],
                                 start=(dc == 0), stop=(dc == DC - 1))
            nc.scalar.copy(v_own[:, pc, :], ps[:, :KF])
        nc.sync.dma_start(vag_in[:].rearrange("(c p) f -> p c f", p=P),
                          v_own[:])
        if solo:
            for r in range(GPB):
                nc.sync.dma_start(vag_out[r * SQ:(r + 1) * SQ, :], vag_in[:])
        else:
            nc.gpsimd.collective_compute(
                "AllGather", mybir.AluOpType.bypass, replica_groups=groups,
                ins=[vag_in[:]], outs=[vag_out[:]])

        # pair 0 consumes wq tiles 2/3 in its own chunk loop - prefetch here
        qproj_mm(1, list(range(DC)),
                 psc.tile([P, 1024], FP, tag="psc", name="q1ps"))
        wq_prefetch(2)
        wq_prefetch(3)

        # ---- land gathered K/V (kT reuses the wk slot); V lands in its
        #      position-major AllGather layout (contiguous 1KB rows - the
        #      head-split layout would pay the sub-512B DMA penalty) ----
        kT = early.tile([P, KFC, S], BF, tag="wkq", name="kT")
        for fc in range(KFC):
            for r in range(GPB):
                nc.scalar.dma_start(kT[:, fc, r * SQ:(r + 1) * SQ],
                                    kag_out[fc, r])
        v_pos = early.tile([P, SC, KF], BF, tag="big32", name="v_pos")
        for c in range(SC):
            nc.sync.dma_start(v_pos[:, c, :], vag_out[c * P:(c + 1) * P, :])

        # ---- per-pair attention loop ----
        oT_tiles = {}

        def wo_load(g, nf):
            wo_nf = opool.tile([P, 4, 512], BF, tag="wo", name="wo_nf")
            nc.sync.dma_start(wo_nf[:], wo_e[g, nf])
            return wo_nf

        out_acc = sb.tile([P, RQ, D], FP)

        def out_proj_m(g, nf, wo_nf, m, slot):
            """One [128-row, 512-col] tile of group g's out-projection,
            accumulated in SBUF; group 3 streams the finished slice out on
            alternating DMA queues."""
            oT = oT_tiles[g]
            ps = pslot(slot, name="ops")
            for ch in range(4):
                nc.tensor.matmul(ps[:],
                                 lhsT=oT[:, ch, m * P:(m + 1) * P],
                                 rhs=wo_nf[:, ch, :],
                                 start=(ch == 0), stop=(ch == 3))
            acc = out_acc[:, m, nf * 512:(nf + 1) * 512]
            if g == 0:
                nc.vector.tensor_copy(acc, ps[:])
            else:
                nc.vector.tensor_add(acc, acc, ps[:])
            if g == 3:
                eng = nc.gpsimd if (nf + m) % 2 == 0 else nc.scalar
                eng.dma_start(
                    out_e[m * P:(m + 1) * P, nf * 512:(nf + 1) * 512], acc)

        fin = {}                # previous pair's normalize/transpose state

        def finish_stage(stage):
            """stage 0: normalize (DVE); 1: PE-transpose; 2: copy to oT."""
            if not fin:
                return
            if stage == 0:
                favA, favB = fin["avA"], fin["avB"]
                o_n = npool.tile([P, RQ, P], BF, tag="onorm", name="o_n")
                rc = npool.tile([P, 8], FP, tag="rc", name="rc")
                nc.vector.reciprocal(rc[:, 0:4], favA[:, :, HD:AW])
                nc.vector.reciprocal(rc[:, 4:8], favB[:, :, HD:AW])
                for blk in range(RQ):
                    tl = favA if blk < 2 else favB
                    for h in range(2):
                        ri = (blk % 2) * 2 + h
                        base = 0 if blk < 2 else 4
                        nc.vector.tensor_scalar_mul(
                            o_n[:, blk, h * HD:(h + 1) * HD],
                            tl[:, ri, 0:HD],
                            rc[:, base + ri:base + ri + 1])
                fin["o_n"] = o_n
            elif stage == 1:
                tp = pslot(1, BF, (P, SQ), name="tp")
                for blk in range(RQ):
                    nc.tensor.transpose(tp[:, blk * P:(blk + 1) * P],
                                        fin["o_n"][:, blk, :], id_sb[:])
                fin["tp"] = tp
            else:
                fg, fpi = fin["pair"] // 4, fin["pair"] % 4
                nc.vector.tensor_copy(oT_tiles[fg][:, fpi, :], fin["tp"][:, 0:SQ])
                fin.clear()

        # qproj injection plan: pairs 0-3 carry two qprojs each (pairs 2-9,
        # second one on slot ppB), pairs 4-9 one each (pairs 10-15).
        qplan = {}
        for p in range(4):
            qplan[p] = (2 * p + 2, 2 * p + 3)
        for p in range(4, 10):
            qplan[p] = (p + 6,)

        for g in range(4):                    # 4 groups of 4 pairs
            oT_tiles[g] = otp.tile([P, RQ, SQ], BF, tag="oT", name=f"oT_{g}")
            for pi in range(4):               # pairs within group
                pair = g * 4 + pi
                wo_cur = [None]
                kc = pair % 4                 # kv chunk holding both kv heads
                kva = 2 * (pair % 4)
                qph = qplan.get(pair, ())

                avA = av.tile([P, 4, AW], FP, tag="av", name="avA")
                avB = av.tile([P, 4, AW], FP, tag="av", name="avB")
                eabs = {}
                for c in range(SC + 2):
                    if c < SC:
                        # scores for both heads of the pair into one 2-bank
                        # psum tile; one exp op covers A and B
                        psAB = psc.tile([P, 1024], FP, tag="psc", name="psAB")
                        nc.tensor.matmul(psAB[:, 0:SQ],
                                         lhsT=kT[0:64, kc, c * P:(c + 1) * P],
                                         rhs=qT[0:64, pair, :],
                                         start=True, stop=True,
                                         tile_position=(0, 0))
                        nc.tensor.matmul(psAB[:, SQ:2 * SQ],
                                         lhsT=kT[64:128, kc, c * P:(c + 1) * P],
                                         rhs=qT[64:128, pair, :],
                                         start=True, stop=True,
                                         tile_position=(64, 0))
                        eab = epool.tile([P, 2, SQ], BF, tag="exp", name="eab")
                        nc.scalar.activation(eab[:], psAB[:], EXPF, scale=EXP_SCALE)
                        eabs[c] = eab
                    # previous pair's normalize/transpose/copy-out
                    if c == 0:
                        finish_stage(0)
                    elif c == 3:
                        finish_stage(1)
                    elif c == 5:
                        finish_stage(2)
                    if c >= 2:
                        cc_ = c - 2      # attn.V lags two chunks behind exp
                        eab = eabs.pop(cc_)
                        # transposed AV: out rows = q positions of one block,
                        # accumulate over kv chunks; a twin N=1 matmul against
                        # the ones column accumulates the softmax denominator
                        # into region col 64.  Region 0 of each bank issues
                        # start (flags the whole bank pending-zero); the rest
                        # ride the flags with start=False.
                        for ti, tl in ((0, avA), (1, avB)):
                            for ri in range(4):
                                blk = ti * 2 + ri // 2
                                h = ri % 2
                                lhsT = eab[:, h, blk * P:(blk + 1) * P]
                                nc.tensor.matmul(
                                    tl[:, ri, 0:HD], lhsT=lhsT,
                                    rhs=v_pos[:, cc_,
                                              (kva + h) * HD:(kva + h + 1) * HD],
                                    start=(cc_ == 0 and ri == 0),
                                    stop=(cc_ == SC - 1),
                                    skip_group_check=True)
                                nc.tensor.matmul(
                                    tl[:, ri, HD:AW], lhsT=lhsT,
                                    rhs=ones1[:],
                                    start=False, stop=(cc_ == SC - 1),
                                    skip_group_check=True)
                    # q-projections: 4 dc-chunks per score chunk keeps ACT fed
                    if len(qph) >= 1 and 0 <= c <= 3:
                        qproj_mm(qph[0], list(range(c * 4, (c + 1) * 4)),
                                 pslot(0, name="qps") if c == 0 else None)
                    if len(qph) >= 2 and 6 <= c <= 9:
                        qproj_mm(qph[1], list(range((c - 6) * 4, (c - 5) * 4)),
                                 pslot(1, name="qps") if c == 6 else None)
                    if c == 5 and g >= 1:
                        wo_cur[0] = wo_load(g - 1, pi)
                    if c == 10 and pair + 1 in qplan:
                        for qp in qplan[pair + 1]:
                            wq_prefetch(qp)
                    if c in (7, 9, 11, 13) and g >= 1:
                        out_proj_m(g - 1, pi, wo_cur[0], (c - 7) // 2,
                                   slot=(1 if c in (7, 11) else 0))
                fin.update({"pair": pair, "avA": avA, "avB": avB})

            if g == 3:
                for st in range(3):
                    finish_stage(st)
                for nf in range(4):
                    wo_nf = wo_load(3, nf)
                    for m in range(RQ):
                        out_proj_m(3, nf, wo_nf, m, slot=m % 2)

    nc.compile()
    return nc


_NC = None


def _get_nc():
    global _NC
    if _NC is None:
        _NC = build()
    return _NC


def _host_prep(inputs):
    """Permute wq/wk to half-rotated layout, swizzle all weights into the
    on-chip layouts (so device DMAs are linear), build CC/SS tables, slice
    per-core shards."""
    x = np.asarray(inputs["x"], np.float32)
    cos = np.asarray(inputs["cos"], np.float32)
    sin = np.asarray(inputs["sin"], np.float32)
    wq = np.asarray(inputs["wq"], np.float32)
    wk = np.asarray(inputs["wk"], np.float32)
    wv = np.asarray(inputs["wv"], np.float32)
    wo = np.asarray(inputs["wo"], np.float32)

    def perm_cols(w, nheads):
        idx = np.empty(nheads * HD, np.int64)
        for h in range(nheads):
            idx[h * HD:h * HD + 32] = h * HD + 2 * np.arange(32)
            idx[h * HD + 32:(h + 1) * HD] = h * HD + 2 * np.arange(32) + 1
        return np.ascontiguousarray(w[:, idx])

    wq_p = perm_cols(wq, NQ)
    wk_p = perm_cols(wk, NKV)
    # device layouts
    BFH = ml_dtypes.bfloat16
    wq_dev = np.ascontiguousarray(
        wq_p.reshape(DC, P, DC, P).transpose(2, 1, 0, 3)).astype(BFH)
    wk_dev = np.ascontiguousarray(
        wk_p.reshape(DC, P, KF).transpose(1, 0, 2)).astype(BFH)
    wv_dev = np.ascontiguousarray(
        wv.reshape(DC, P, KF).transpose(1, 0, 2)).astype(BFH)
    wo_dev = np.ascontiguousarray(
        wo.reshape(RQ, RQ, P, RQ, 512).transpose(0, 3, 2, 1, 4)).astype(BFH)

    cosT = np.ascontiguousarray(cos.T)            # [32, S]
    sinT = np.ascontiguousarray(sin.T)
    CC = np.tile(cosT, (4, 1))                    # [128, S]
    SS = np.concatenate([-sinT, sinT, -sinT, sinT], 0)

    in_maps = []
    for c in range(NCORES):
        b, q = c // GPB, c % GPB
        sl = slice(q * SQ, (q + 1) * SQ)
        x_dev = np.ascontiguousarray(
            x[b, sl, :].T.reshape(DC, P, SQ).transpose(1, 0, 2)).astype(
                ml_dtypes.bfloat16)
        in_maps.append({
            "x": x_dev,
            "wq": wq_dev, "wk": wk_dev, "wv": wv_dev, "wo": wo_dev,
            "cc": np.ascontiguousarray(CC[:, sl]),
            "ss": np.ascontiguousarray(SS[:, sl]),
        })
    return in_maps


def kernel(**inputs):
    nc = _get_nc()
    in_maps = _host_prep(inputs)
    res = run_bass_kernel_spmd(nc, in_maps, core_ids=list(range(NCORES)))
    out = np.empty((B, S, D), np.float32)
    for c in range(NCORES):
        b, q = c // GPB, c % GPB
        out[b, q * SQ:(q + 1) * SQ, :] = res.results[c]["out"]
    return out


# revision 22
# speedup vs baseline: 1.1830x; 1.0142x over previous
"""Distributed GQA attention (llama-style RoPE) for one TRN2 chip (8 NeuronCores).

Sharding: core c handles batch b=c//4 and sequence-quarter q=c%4 (512 q-rows).
Each core projects Q for its own rows (all 32 heads), projects K/V for its own
512 positions, AllGathers K/V within its 4-core batch group, runs attention for
its rows, and applies the output projection. Output rows are disjoint across
cores, so no all-reduce is needed; the host concatenates.

On-chip dataflow (per core):
  xT   = x.T (pre-transposed on host, landed in 4 column chunks)  [d, rows]
  kT   = wk.T @ xT  -> RoPE -> bf16 -> AllGather (one per feature chunk,
         dispatched as soon as that chunk's rope is done) -> [feat, skv].
         kproj runs FIRST so the ~267us exp stream can start early.
  v    = xT.T @ wv  -> bf16 (PSUM->SBUF copy on the idle ACT engine)
         -> AllGather -> [skv, feat]
  per head pair:
    qT   = wq.T @ xT -> RoPE -> bf16                 [feat, sq]
    sT   = kT_h.T @ qT_h  (row-packed pairs)         [skv, sq]  psum f32
    e    = exp(sT/8) on ScalarE -> bf16
    oTr  = e_blk.T @ [v|1]  accum over skv chunks    [sq, 65]   psum f32
           (transposed AV: out rows = q positions, col 64 = softmax denom;
            costs 65 moving rows/chunk vs 2x512 for the straight form)
    o_n  = oTr[:, :64] * recip(oTr[:, 64])  -> bf16  [sq, feat]
    oT   = PE-transpose(o_n)                         [feat, sq]
  out  = oT.T @ wo, 4 head-group partials accumulated in SBUF via DVE
         (GPSIMD cannot access PSUM on TRN2), streamed to DRAM on two queues.

Schedule: every engine queue executes IN ORDER, so instruction emission order
is the schedule.  Per pair (17-step chunk loop, attn.V lagging 2 chunks):
  c0-c3 : next q-projection, 4 dc-chunks per step (pairs 0-3 carry a second
          q-projection at c6-c9 - they have no out-proj to interleave)
  c0/c3/c5: previous pair's normalize / PE-transpose / copy-to-oT, placed so
          the DVE normalize is certainly done before the PE transposes issue
  c7,9,11,13: out-proj quarters of the previous group (+ c5: wo prefetch,
          c10: wq prefetch for the next pair's q-projection)
PSUM (8 banks): scores/exp 2 bufs x 2 banks; two explicitly-addressed 1-bank
slots ppA (q-proj, even out-proj tiles) and ppB (transposes, odd out-proj
tiles, second q-projection) - explicit tags, not round-robin, so no request
ever queues behind a slot whose consumer hasn't run yet; AV accum 2 x 1 bank.
The AV banks hold 4 accumulation regions each ([128,65] f32); only the first
region per bank issues start=True (flags the whole 2KB bank pending-zero),
later regions ride the bank flags with start=False - in-order PE issue makes
this safe.

All weights are pre-swizzled on the HOST into the exact SBUF layouts so every
DMA is a fully-linear copy. RoPE uses the half-rotated layout: wq/wk columns
are permuted on the host so each head's features are [evens(32) | odds(32)];
cos/sin tables are shipped pre-transposed/tiled as CC/SS [128, sq].
"""
import sys

sys.path.insert(0, "/opt/trn_rl_repo")

import numpy as np
import ml_dtypes
from contextlib import ExitStack

import concourse.bass as bass
import concourse.mybir as mybir
import concourse.tile as tile
from concourse import bacc
from concourse.bass_utils import run_bass_kernel_spmd
from concourse.masks import make_identity

B, S, D = 2, 2048, 2048
NQ, NKV, HD = 32, 8, 64
NCORES = 8
GPB = 4                 # cores per batch group
SQ = S // GPB           # 512 q-rows per core
P = 128
DC = D // P             # 16 contraction chunks
KF = NKV * HD           # 512 kv feature dim
KFC = KF // P           # 4 kv feature chunks
SC = S // P             # 16 skv chunks
RQ = SQ // P            # 4 q-row blocks
AW = HD + 1             # AV width: 64 v cols + 1 ones col (softmax denom)

FP = mybir.dt.float32
BF = mybir.dt.bfloat16
EXPF = mybir.ActivationFunctionType.Exp
EXP_SCALE = 1.0 / 8.0   # 1/sqrt(HD)


def build(solo=False):
    nc = bacc.Bacc("TRN2", target_bir_lowering=False, debug=False,
                   num_devices=1 if solo else NCORES)

    x_e = nc.dram_tensor("x", [P, DC, SQ], BF, kind="ExternalInput").ap()
    wq_e = nc.dram_tensor("wq", [DC, P, DC, P], BF, kind="ExternalInput").ap()
    wk_e = nc.dram_tensor("wk", [KFC, P, DC, P], BF, kind="ExternalInput").ap()
    wv_e = nc.dram_tensor("wv", [P, DC, KF], BF, kind="ExternalInput").ap()
    wo_e = nc.dram_tensor("wo", [RQ, RQ, P, RQ, 512], BF, kind="ExternalInput").ap()
    cc_e = nc.dram_tensor("cc", [P, SQ], FP, kind="ExternalInput").ap()
    ss_e = nc.dram_tensor("ss", [P, SQ], FP, kind="ExternalInput").ap()
    out_e = nc.dram_tensor("out", [SQ, D], FP, kind="ExternalOutput").ap()

    groups = [[0, 1, 2, 3], [4, 5, 6, 7]]

    with tile.TileContext(nc) as tc, ExitStack() as ctx:
        sb = ctx.enter_context(tc.tile_pool(name="sb", bufs=1))
        rp = ctx.enter_context(tc.tile_pool(name="rp", bufs=3))
        epool = ctx.enter_context(tc.tile_pool(name="epool", bufs=6))
        npool = ctx.enter_context(tc.tile_pool(name="npool", bufs=3))
        opool = ctx.enter_context(tc.tile_pool(name="opool", bufs=4))
        otp = ctx.enter_context(tc.tile_pool(name="otp", bufs=2))
        early = ctx.enter_context(tc.tile_pool(name="early", bufs=1))
        wqp = ctx.enter_context(tc.tile_pool(name="wqp", bufs=4))
        dram = ctx.enter_context(tc.tile_pool(name="dram", bufs=1, space="DRAM"))
        pp = ctx.enter_context(tc.tile_pool(name="pp", bufs=1, space="PSUM"))
        psc = ctx.enter_context(tc.tile_pool(name="psc", bufs=2, space="PSUM"))
        av = ctx.enter_context(tc.tile_pool(name="av", bufs=2, space="PSUM"))

        # ---- constants ----
        cc_sb = sb.tile([P, SQ], FP)
        ss_sb = sb.tile([P, SQ], FP)
        id_sb = sb.tile([P, P], BF)

        def rope_chunk(ps, dst):
            """dst = RoPE(ps) in half-rotated layout; ps [128,SQ] psum f32."""
            t0 = rp.tile([P, SQ], FP, tag="t0")
            t1 = rp.tile([P, SQ], FP, tag="t1")
            nc.vector.tensor_mul(t0[:], ps[:], cc_sb[:])
            for blk in range(4):
                o0, i0 = blk * 32, (blk ^ 1) * 32
                nc.vector.tensor_mul(t1[o0:o0 + 32, :], ps[i0:i0 + 32, :],
                                     ss_sb[o0:o0 + 32, :])
            nc.vector.tensor_add(dst, t0[:], t1[:])

        qT = sb.tile([P, DC, SQ], BF)
        kag_in = dram.tile([KFC, P, SQ], BF)
        kag_out = dram.tile([KFC, GPB, P, SQ], BF)
        vag_in = dram.tile([RQ, P, KF], BF)
        vag_out = dram.tile([RQ, GPB, P, KF], BF)

        # ---- input loads: the DMA bus is one shared 360GB/s resource in
        #      arrival order, so emit the bytes the pipeline needs first:
        #      x/wk in interleaved quarter-chunks (kproj contracts as they
        #      land), then rope tables, then wq0, then wv ----
        xT = early.tile([P, DC, SQ], BF, tag="xT", name="xT")
        wkq = [early.tile([P, DC, P], BF, tag="wkqf", bufs=KFC,
                          name=f"wkq{fc}") for fc in range(KFC)]
        for qc in range(4):
            nc.sync.dma_start(xT[:, qc * 4:(qc + 1) * 4, :],
                              x_e[:, qc * 4:(qc + 1) * 4, :])
            nc.scalar.dma_start(wkq[qc][:], wk_e[qc])
            if qc == 0:
                nc.sync.dma_start(cc_sb[:], cc_e)
                nc.sync.dma_start(ss_sb[:], ss_e)
        wq_tiles = {}

        def wq_prefetch(pair):
            w = wqp.tile([P, DC, P], BF, tag="wq", name=f"wq_{pair}")
            nc.sync.dma_start(w[:], wq_e[pair])
            wq_tiles[pair] = w

        wq_prefetch(0)
        wv_sb = early.tile([P, DC, KF], BF, tag="big32", name="wv_sb")
        nc.sync.dma_start(wv_sb[:], wv_e)
        wq_prefetch(1)
        make_identity(nc, id_sb)
        ones1 = sb.tile([P, 1], BF)
        nc.gpsimd.memset(ones1[:], 1.0)

        def pslot(slot, dtype=FP, shape=(P, 512), name="ps"):
            return pp.tile(list(shape), dtype, tag=f"pp{slot}", name=name)

        qproj_ps = {}

        def qproj_mm(pair, dcs, qps):
            """Emit dc-chunk matmuls of the Q projection for `pair`; rope and
            release the psum slot after the last chunk."""
            if qps is not None:
                qproj_ps[pair] = (wq_tiles.pop(pair), qps)
            wq_sb, qps = qproj_ps[pair]
            for dc in dcs:
                nc.tensor.matmul(qps[:, :SQ], lhsT=wq_sb[:, dc, :],
                                 rhs=xT[:, dc, :],
                                 start=(dc == 0), stop=(dc == DC - 1))
            if dcs[-1] == DC - 1:
                rope_chunk(qps[:, :SQ], qT[:, pair, :])
                del qproj_ps[pair]

        # ---- K projection + RoPE first; AllGather per feature chunk so the
        #      gather pipeline overlaps the remaining ropes.  qproj(0) sits
        #      between the kproj halves: its rope (which gates pair-0 scores)
        #      lands 3rd in the DVE chain, while AllGathers for the late kv
        #      chunks (first needed by pairs 2/3) may finish late ----
        kT_own = sb.tile([P, KFC, SQ], BF, tag="own4", name="kT_own")

        def kproj(fc):
            ps = pslot(fc % 2, name="kps")
            for dc in range(DC):
                nc.tensor.matmul(ps[:, :SQ],
                                 lhsT=wkq[fc][:, dc, :],
                                 rhs=xT[:, dc, :],
                                 start=(dc == 0), stop=(dc == DC - 1))
            rope_chunk(ps[:, :SQ], kT_own[:, fc, :])
            nc.sync.dma_start(kag_in[fc], kT_own[:, fc, :])
            if solo:
                for r in range(GPB):
                    nc.sync.dma_start(kag_out[fc, r], kag_in[fc])
            else:
                nc.gpsimd.collective_compute(
                    "AllGather", mybir.AluOpType.bypass,
                    replica_groups=groups,
                    ins=[kag_in[fc]], outs=[kag_out[fc]])

        kproj(0)
        kproj(1)
        qproj_mm(0, list(range(DC)),
                 psc.tile([P, 1024], FP, tag="psc", name="q0ps"))
        kproj(2)
        kproj(3)

        # ---- V projection -> AllGather, one per 128-row block so the first
        #      kv chunks reach the attention loop as early as possible.
        #      PSUM: the av-pool banks (idle until pair 0's AV) - the pp
        #      slots are WAR-held by the k-ropes; copies on ACT (DVE busy) ----
        v_own = sb.tile([P, RQ, KF], BF, tag="own4", name="v_own")
        for pc in range(RQ):
            ps = av.tile([P, 512], FP, tag="av", name="vps")
            for dc in range(DC):
                nc.tensor.matmul(ps[:, :KF],
                                 lhsT=xT[:, dc, pc * P:(pc + 1) * P],
                                 rhs=wv_sb[:, dc, :],
                                 start=(dc == 0), stop=(dc == DC - 1))
            nc.scalar.copy(v_own[:, pc, :], ps[:, :KF])
            nc.sync.dma_start(vag_in[pc], v_own[:, pc, :])
            if solo:
                for r in range(GPB):
                    nc.sync.dma_start(vag_out[pc, r], vag_in[pc])
            else:
                nc.gpsimd.collective_compute(
                    "AllGather", mybir.AluOpType.bypass,
                    replica_groups=groups,
                    ins=[vag_in[pc]], outs=[vag_out[pc]])

        # pair 0 consumes wq tiles 2/3 in its own chunk loop - prefetch here
        qproj_mm(1, list(range(DC)),
                 psc.tile([P, 1024], FP, tag="psc", name="q1ps"))
        wq_prefetch(2)
        wq_prefetch(3)

        # ---- land gathered K/V (kT reuses the wk slot); V lands in its
        #      position-major AllGather layout (contiguous 1KB rows - the
        #      head-split layout would pay the sub-512B DMA penalty) ----
        kT = early.tile([P, KFC, S], BF, tag="kT", name="kT")
        for fc in range(KFC):
            for r in range(GPB):
                nc.scalar.dma_start(kT[:, fc, r * SQ:(r + 1) * SQ],
                                    kag_out[fc, r])
        v_pos = early.tile([P, SC, KF], BF, tag="big32", name="v_pos")
        for c in range(SC):
            nc.sync.dma_start(v_pos[:, c, :], vag_out[c % 4, c // 4])

        # ---- per-pair attention loop ----
        oT_tiles = {}

        def wo_load(g, nf):
            wo_nf = opool.tile([P, 4, 512], BF, tag="wo", name="wo_nf")
            nc.sync.dma_start(wo_nf[:], wo_e[g, nf])
            return wo_nf

        out_acc = sb.tile([P, RQ, D], FP)

        def out_proj_m(g, nf, wo_nf, m, slot):
            """One [128-row, 512-col] tile of group g's out-projection,
            accumulated in SBUF; group 3 streams the finished slice out on
            alternating DMA queues."""
            oT = oT_tiles[g]
            ps = pslot(slot, name="ops")
            for ch in range(4):
                nc.tensor.matmul(ps[:],
                                 lhsT=oT[:, ch, m * P:(m + 1) * P],
                                 rhs=wo_nf[:, ch, :],
                                 start=(ch == 0), stop=(ch == 3))
            acc = out_acc[:, m, nf * 512:(nf + 1) * 512]
            if g == 0:
                nc.vector.tensor_copy(acc, ps[:])
            else:
                nc.vector.tensor_add(acc, acc, ps[:])
            if g == 3:
                eng = nc.gpsimd if (nf + m) % 2 == 0 else nc.scalar
                eng.dma_start(
                    out_e[m * P:(m + 1) * P, nf * 512:(nf + 1) * 512], acc)

        fin = {}                # previous pair's normalize/transpose state

        def finish_stage(stage):
            """stage 0: normalize (DVE); 1: PE-transpose; 2: copy to oT."""
            if not fin:
                return
            if stage == 0:
                favA, favB = fin["avA"], fin["avB"]
                o_n = npool.tile([P, RQ, P], BF, tag="onorm", name="o_n")
                rc = npool.tile([P, 8], FP, tag="rc", name="rc")
                nc.vector.reciprocal(rc[:, 0:4], favA[:, :, HD:AW])
                nc.vector.reciprocal(rc[:, 4:8], favB[:, :, HD:AW])
                for blk in range(RQ):
                    tl = favA if blk < 2 else favB
                    for h in range(2):
                        ri = (blk % 2) * 2 + h
                        base = 0 if blk < 2 else 4
                        nc.vector.tensor_scalar_mul(
                            o_n[:, blk, h * HD:(h + 1) * HD],
                            tl[:, ri, 0:HD],
                            rc[:, base + ri:base + ri + 1])
                fin["o_n"] = o_n
            elif stage == 1:
                tp = pslot(1, BF, (P, SQ), name="tp")
                for blk in range(RQ):
                    nc.tensor.transpose(tp[:, blk * P:(blk + 1) * P],
                                        fin["o_n"][:, blk, :], id_sb[:])
                fin["tp"] = tp
            else:
                fg, fpi = fin["pair"] // 4, fin["pair"] % 4
                nc.vector.tensor_copy(oT_tiles[fg][:, fpi, :], fin["tp"][:, 0:SQ])
                fin.clear()

        # qproj injection plan: pairs 0-3 carry two qprojs each (pairs 2-9,
        # second one on slot ppB), pairs 4-9 one each (pairs 10-15).
        qplan = {}
        for p in range(4):
            qplan[p] = (2 * p + 2, 2 * p + 3)
        for p in range(4, 10):
            qplan[p] = (p + 6,)

        for g in range(4):                    # 4 groups of 4 pairs
            oT_tiles[g] = otp.tile([P, RQ, SQ], BF, tag="oT", name=f"oT_{g}")
            for pi in range(4):               # pairs within group
                pair = g * 4 + pi
                wo_cur = [None]
                kc = pair % 4                 # kv chunk holding both kv heads
                kva = 2 * (pair % 4)
                qph = qplan.get(pair, ())

                avA = av.tile([P, 4, AW], FP, tag="av", name="avA")
                avB = av.tile([P, 4, AW], FP, tag="av", name="avB")
                eabs = {}
                for c in range(SC + 2):
                    if c < SC:
                        # scores for both heads of the pair into one 2-bank
                        # psum tile; one exp op covers A and B
                        psAB = psc.tile([P, 1024], FP, tag="psc", name="psAB")
                        nc.tensor.matmul(psAB[:, 0:SQ],
                                         lhsT=kT[0:64, kc, c * P:(c + 1) * P],
                                         rhs=qT[0:64, pair, :],
                                         start=True, stop=True,
                                         tile_position=(0, 0))
                        nc.tensor.matmul(psAB[:, SQ:2 * SQ],
                                         lhsT=kT[64:128, kc, c * P:(c + 1) * P],
                                         rhs=qT[64:128, pair, :],
                                         start=True, stop=True,
                                         tile_position=(64, 0))
                        eab = epool.tile([P, 2, SQ], BF, tag="exp", name="eab")
                        nc.scalar.activation(eab[:], psAB[:], EXPF, scale=EXP_SCALE)
                        eabs[c] = eab
                    # previous pair's normalize/transpose/copy-out
                    if c == 0:
                        finish_stage(0)
                    elif c == 3:
                        finish_stage(1)
                    elif c == 5:
                        finish_stage(2)
                    if c >= 2:
                        cc_ = c - 2      # attn.V lags two chunks behind exp
                        eab = eabs.pop(cc_)
                        # transposed AV: out rows = q positions of one block,
                        # accumulate over kv chunks; a twin N=1 matmul against
                        # the ones column accumulates the softmax denominator
                        # into region col 64.  Region 0 of each bank issues
                        # start (flags the whole bank pending-zero); the rest
                        # ride the flags with start=False.
                        for ti, tl in ((0, avA), (1, avB)):
                            for ri in range(4):
                                blk = ti * 2 + ri // 2
                                h = ri % 2
                                lhsT = eab[:, h, blk * P:(blk + 1) * P]
                                nc.tensor.matmul(
                                    tl[:, ri, 0:HD], lhsT=lhsT,
                                    rhs=v_pos[:, cc_,
                                              (kva + h) * HD:(kva + h + 1) * HD],
                                    start=(cc_ == 0 and ri == 0),
                                    stop=(cc_ == SC - 1),
                                    skip_group_check=True)
                                nc.tensor.matmul(
                                    tl[:, ri, HD:AW], lhsT=lhsT,
                                    rhs=ones1[:],
                                    start=False, stop=(cc_ == SC - 1),
                                    skip_group_check=True)
                    # q-projections: 4 dc-chunks per score chunk keeps ACT fed
                    if len(qph) >= 1 and 0 <= c <= 3:
                        qproj_mm(qph[0], list(range(c * 4, (c + 1) * 4)),
                                 pslot(0, name="qps") if c == 0 else None)
                    if len(qph) >= 2 and 6 <= c <= 9:
                        qproj_mm(qph[1], list(range((c - 6) * 4, (c - 5) * 4)),
                                 pslot(1, name="qps") if c == 6 else None)
                    if c == 5 and g >= 1:
                        wo_cur[0] = wo_load(g - 1, pi)
                    if c == 10 and pair + 1 in qplan:
                        for qp in qplan[pair + 1]:
                            wq_prefetch(qp)
                    if c in (7, 9, 11, 13) and g >= 1:
                        out_proj_m(g - 1, pi, wo_cur[0], (c - 7) // 2,
                                   slot=(1 if c in (7, 11) else 0))
                fin.update({"pair": pair, "avA": avA, "avB": avB})

            if g == 3:
                for st in range(3):
                    finish_stage(st)
                for nf in range(4):
                    wo_nf = wo_load(3, nf)
                    for m in range(RQ):
                        out_proj_m(3, nf, wo_nf, m, slot=m % 2)

    nc.compile()
    return nc


_NC = None


def _get_nc():
    global _NC
    if _NC is None:
        _NC = build()
    return _NC


def _host_prep(inputs):
    """Permute wq/wk to half-rotated layout, swizzle all weights into the
    on-chip layouts (so device DMAs are linear), build CC/SS tables, slice
    per-core shards."""
    x = np.asarray(inputs["x"], np.float32)
    cos = np.asarray(inputs["cos"], np.float32)
    sin = np.asarray(inputs["sin"], np.float32)
    wq = np.asarray(inputs["wq"], np.float32)
    wk = np.asarray(inputs["wk"], np.float32)
    wv = np.asarray(inputs["wv"], np.float32)
    wo = np.asarray(inputs["wo"], np.float32)

    def perm_cols(w, nheads):
        idx = np.empty(nheads * HD, np.int64)
        for h in range(nheads):
            idx[h * HD:h * HD + 32] = h * HD + 2 * np.arange(32)
            idx[h * HD + 32:(h + 1) * HD] = h * HD + 2 * np.arange(32) + 1
        return np.ascontiguousarray(w[:, idx])

    wq_p = perm_cols(wq, NQ)
    wk_p = perm_cols(wk, NKV)
    # device layouts
    BFH = ml_dtypes.bfloat16
    wq_dev = np.ascontiguousarray(
        wq_p.reshape(DC, P, DC, P).transpose(2, 1, 0, 3)).astype(BFH)
    wk_dev = np.ascontiguousarray(
        wk_p.reshape(DC, P, KFC, P).transpose(2, 1, 0, 3)).astype(BFH)
    wv_dev = np.ascontiguousarray(
        wv.reshape(DC, P, KF).transpose(1, 0, 2)).astype(BFH)
    wo_dev = np.ascontiguousarray(
        wo.reshape(RQ, RQ, P, RQ, 512).transpose(0, 3, 2, 1, 4)).astype(BFH)

    cosT = np.ascontiguousarray(cos.T)            # [32, S]
    sinT = np.ascontiguousarray(sin.T)
    CC = np.tile(cosT, (4, 1))                    # [128, S]
    SS = np.concatenate([-sinT, sinT, -sinT, sinT], 0)

    in_maps = []
    for c in range(NCORES):
        b, q = c // GPB, c % GPB
        sl = slice(q * SQ, (q + 1) * SQ)
        x_dev = np.ascontiguousarray(
            x[b, sl, :].T.reshape(DC, P, SQ).transpose(1, 0, 2)).astype(
                ml_dtypes.bfloat16)
        in_maps.append({
            "x": x_dev,
            "wq": wq_dev, "wk": wk_dev, "wv": wv_dev, "wo": wo_dev,
            "cc": np.ascontiguousarray(CC[:, sl]),
            "ss": np.ascontiguousarray(SS[:, sl]),
        })
    return in_maps


def kernel(**inputs):
    nc = _get_nc()
    in_maps = _host_prep(inputs)
    res = run_bass_kernel_spmd(nc, in_maps, core_ids=list(range(NCORES)))
    out = np.empty((B, S, D), np.float32)
    for c in range(NCORES):
        b, q = c // GPB, c % GPB
        out[b, q * SQ:(q + 1) * SQ, :] = res.results[c]["out"]
    return out
